# revision 18
# baseline (speedup 1.0000x reference)
"""CRF loss (negative log-likelihood) kernel for Trainium2, 8 NeuronCores.

Strategy (data-parallel over batch, per sharding hint):
  - Each of 8 cores gets B/8 = 64 sequences. Same NEFF on all cores (SPMD).
  - Denominator (log partition) via exp-domain forward-backward:
      fwd:  p_i = (E^T p_{i-1}) * x_i          (matmul on PE + multiply on DVE)
      bwd:  w_{j-1} = x_{j-1} * (E w_j)
      meet at i=256:  logZ_b = ln(sum_t p_256 * v_256) + c_f + c_b
    where E = exp(transitions), x_i = exp(emissions_i) in (tag, batch) layout.
    Running the two recursions concurrently halves the serial latency chain.
    Periodic renormalization (every 64 steps) keeps values in fp32/bf16 range;
    the applied scale is a bf16 value whose exact ln is accumulated, so the
    scaling cancels exactly.
  - Emissions stream: SWDGE DMA casts fp32->bf16 into a (2x64, t) stacked
    layout, HWDGE xbar transpose-DMA produces (t, b) tiles, ACT exps them in
    512-wide batches.
  - Numerator (gold path score) via indirect DMA gathers: emissions at gold
    tags, transitions at tag pairs, start/end transitions; reduced on device,
    summed on host.
"""

import os
import sys

import numpy as np

for _p in ("/opt/trn_rl_repo", os.path.expanduser("~/.axon_site/_ro/trn_rl_repo")):
    if os.path.isdir(_p):
        if _p not in sys.path:
            sys.path.insert(0, _p)
        break

import concourse.bass as bass  # noqa: E402
from concourse import mybir  # noqa: E402
from concourse.masks import make_identity  # noqa: E402
from concourse.tile import TileContext  # noqa: E402

FP32 = mybir.dt.float32
BF16 = mybir.dt.bfloat16
I32 = mybir.dt.int32
Exp = mybir.ActivationFunctionType.Exp
Ln = mybir.ActivationFunctionType.Ln
Add = mybir.AluOpType.add
Sub = mybir.AluOpType.subtract
Mult = mybir.AluOpType.mult

L, B, T = 512, 512, 128
NCORES = 8
BL = B // NCORES  # 64 sequences per core


def build_crf_kernel(L=L, BL=BL, T=T, CH=32, RENORM=64):
    """Build the per-core Bass kernel (SPMD: same NEFF, different inputs)."""
    assert L % CH == 0 and CH % 2 == 0
    nchunks = L // CH
    MID = L // 2  # fwd covers steps 1..MID, bwd covers MID+1..L-1
    TCH = min(128, L)  # tags chunk (steps on partitions)
    ntch = (L + TCH - 1) // TCH
    GW = BL  # free width contributed per tags chunk in the gather tiles

    nc = bass.Bass()

    emis = nc.declare_dram_parameter("emissions", [L, BL, T], FP32, isOutput=False)
    tags = nc.declare_dram_parameter("tags", [L, BL], I32, isOutput=False)
    start_t = nc.declare_dram_parameter("start_t", [T, 1], FP32, isOutput=False)
    end_t = nc.declare_dram_parameter("end_t", [T, 1], FP32, isOutput=False)
    trans = nc.declare_dram_parameter("trans", [T, T], FP32, isOutput=False)
    out_z = nc.declare_dram_parameter("out_z", [1, BL], FP32, isOutput=True)
    out_gold = nc.declare_dram_parameter("out_gold", [TCH, 1], FP32, isOutput=True)
    out_trans = nc.declare_dram_parameter("out_trans", [TCH, 1], FP32, isOutput=True)
    out_se = nc.declare_dram_parameter("out_se", [16, 8], FP32, isOutput=True)

    from contextlib import ExitStack

    with TileContext(nc) as tc, ExitStack() as es:
        cpool = es.enter_context(tc.tile_pool(name="consts", bufs=1))
        ebf_pool = es.enter_context(tc.tile_pool(name="ebf", bufs=2))
        xtr_pool = es.enter_context(tc.tile_pool(name="xtraw", bufs=2))
        xf_pool = es.enter_context(tc.tile_pool(name="x_f", bufs=3))
        xb_pool = es.enter_context(tc.tile_pool(name="x_b", bufs=3))
        p_pool = es.enter_context(tc.tile_pool(name="pp", bufs=4))
        sm_pool = es.enter_context(tc.tile_pool(name="small", bufs=2))
        num_pool = es.enter_context(tc.tile_pool(name="numer", bufs=1))
        tg_pool = es.enter_context(tc.tile_pool(name="tagt", bufs=2))
        ps_q = es.enter_context(tc.tile_pool(name="ps_q", bufs=2, space="PSUM"))
        ps_misc = es.enter_context(tc.tile_pool(name="ps_misc", bufs=1, space="PSUM"))

        # ---------------- constants ----------------
        trans_sb = cpool.tile([T, T], FP32, tag="trans_sb")
        nc.sync.dma_start(out=trans_sb[:], in_=trans[:])
        ident = cpool.tile([128, 128], FP32, tag="ident")
        make_identity(nc, ident[:])
        # Fold a 2^-7 scale into E so per-step mass growth is ~1 (the
        # sum over 128 source tags would otherwise overflow in ~16 steps).
        # Compensated exactly by +(L-1)*7*ln2 on the final log-partition.
        LOG_SCALE = -7.0 * float(np.log(2.0))
        lsc_col = cpool.tile([128, 1], FP32, tag="lsc_col")
        nc.vector.memset(lsc_col[:], LOG_SCALE)
        E_bf = cpool.tile([T, T], BF16, tag="E_bf")
        nc.scalar.activation(out=E_bf[:], in_=trans_sb[:], func=Exp, bias=lsc_col[:])
        transT_ps = ps_misc.tile([T, T], FP32, space="PSUM", tag="transT")
        nc.tensor.transpose(out=transT_ps[:], in_=trans_sb[:], identity=ident[:])
        ET_bf = cpool.tile([T, T], BF16, tag="ET_bf")
        nc.scalar.activation(
            out=ET_bf[:], in_=transT_ps[:], func=Exp, bias=lsc_col[:]
        )

        start_col = cpool.tile([T, 1], FP32, tag="start_col")
        nc.sync.dma_start(out=start_col[:], in_=start_t[:])
        end_col = cpool.tile([T, 1], FP32, tag="end_col")
        nc.sync.dma_start(out=end_col[:], in_=end_t[:])

        ones_col_bf = cpool.tile([128, 1], BF16, tag="ones_col_bf")
        nc.vector.memset(ones_col_bf[:], 1.0)
        ones_col_f32 = cpool.tile([128, 1], FP32, tag="ones_col_f32")
        nc.vector.memset(ones_col_f32[:], 1.0)
        ones_row_bf = cpool.tile([1, 128], BF16, tag="ones_row_bf")
        nc.vector.memset(ones_row_bf[:], 1.0)
        ones_bl_bf = cpool.tile([128, BL], BF16, tag="ones_bl_bf")
        nc.vector.memset(ones_bl_bf[:], 1.0)

        c_f = sm_pool.tile([1, BL], FP32, tag="c_f")
        nc.vector.memset(c_f[:], 0.0)
        c_b = sm_pool.tile([1, BL], FP32, tag="c_b")
        nc.vector.memset(c_b[:], 0.0)

        # ---------------- numerator: gathers ----------------
        gold_idx = num_pool.tile([TCH, L * BL // TCH], I32, tag="gold_idx")
        trans_idx = num_pool.tile([TCH, L * BL // TCH], I32, tag="trans_idx")
        tags_cur = {}
        for c in range(ntch):
            tcur = tg_pool.tile([TCH, BL], I32, tag="tags_cur")
            nc.sync.dma_start(out=tcur[:], in_=tags[c * TCH:(c + 1) * TCH, :])
            tags_cur[c] = tcur
            gsl = gold_idx[:, c * GW:(c + 1) * GW]
            # gold flat index = (i*BL + b)*T + tags[i, b]
            nc.gpsimd.iota(
                gsl, pattern=[[T, BL]], base=c * TCH * BL * T,
                channel_multiplier=BL * T,
            )
            nc.vector.tensor_tensor(out=gsl, in0=gsl, in1=tcur[:], op=Add)

            tprev = tg_pool.tile([TCH, BL], I32, tag="tags_prev")
            if c == 0:
                nc.vector.memset(tprev[0:1, :], 0)
                nc.sync.dma_start(out=tprev[1:TCH, :], in_=tags[0:TCH - 1, :])
            else:
                nc.sync.dma_start(
                    out=tprev[:], in_=tags[c * TCH - 1:(c + 1) * TCH - 1, :]
                )
            tsl = trans_idx[:, c * GW:(c + 1) * GW]
            # trans flat index = tags[i-1]*T + tags[i]
            nc.vector.tensor_scalar(
                out=tsl, in0=tprev[:], scalar1=T, scalar2=None, op0=Mult
            )
            nc.vector.tensor_tensor(out=tsl, in0=tsl, in1=tcur[:], op=Add)
        # pair step 0 does not exist: poison its indices; bounds_check skips them
        nc.vector.memset(trans_idx[0:1, 0:GW], 1 << 24)

        gvals = num_pool.tile([TCH, L * BL // TCH], FP32, tag="gvals")
        nc.gpsimd.indirect_dma_start(
            out=gvals[:], out_offset=None, in_=emis[:],
            in_offset=bass.IndirectOffsetOnAxis(ap=gold_idx[:], axis=2),
            bounds_check=L * BL * T - 1, oob_is_err=False,
        )
        tvals = num_pool.tile([TCH, L * BL // TCH], FP32, tag="tvals")
        nc.gpsimd.indirect_dma_start(
            out=tvals[:], out_offset=None, in_=trans[:],
            in_offset=bass.IndirectOffsetOnAxis(ap=trans_idx[:], axis=1),
            bounds_check=T * T - 1, oob_is_err=False,
        )
        gold_red = num_pool.tile([TCH, 1], FP32, tag="gold_red")
        nc.vector.tensor_reduce(
            out=gold_red[:], in_=gvals[:], axis=mybir.AxisListType.X, op=Add
        )
        trans_red = num_pool.tile([TCH, 1], FP32, tag="trans_red")
        nc.vector.tensor_reduce(
            out=trans_red[:], in_=tvals[:], axis=mybir.AxisListType.X, op=Add
        )
        nc.sync.dma_start(out=out_gold[:], in_=gold_red[:])
        nc.sync.dma_start(out=out_trans[:], in_=trans_red[:])

        # start/end transition gathers (64 each)
        se_idx = num_pool.tile([16, 8], I32, tag="se_idx")
        nc.sync.dma_start(
            out=se_idx[:, 0:4], in_=tags[0:1, :].rearrange("o (p j) -> (o p) j", p=16)
        )
        nc.sync.dma_start(
            out=se_idx[:, 4:8],
            in_=tags[L - 1:L, :].rearrange("o (p j) -> (o p) j", p=16),
        )
        se_vals = num_pool.tile([16, 8], FP32, tag="se_vals")
        nc.gpsimd.indirect_dma_start(
            out=se_vals[:, 0:4], out_offset=None, in_=start_t[:],
            in_offset=bass.IndirectOffsetOnAxis(ap=se_idx[:, 0:4], axis=1),
            bounds_check=T - 1, oob_is_err=False,
        )
        nc.gpsimd.indirect_dma_start(
            out=se_vals[:, 4:8], out_offset=None, in_=end_t[:],
            in_offset=bass.IndirectOffsetOnAxis(ap=se_idx[:, 4:8], axis=1),
            bounds_check=T - 1, oob_is_err=False,
        )
        nc.sync.dma_start(out=out_se[:], in_=se_vals[:])

        # ---------------- emissions stream: cast + transpose + exp ----------------
        H = CH // 2
        x_tiles = {}      # chunk -> x tile (exp'ed, (t, b) layout)
        xtraw_tiles = {}  # chunk -> pre-exp transposed tile (for biased inits)

        def emit_chunk(c, pool):
            ebf = ebf_pool.tile([2 * BL, H * T], BF16, tag="ebf")
            for h in range(2):
                src = emis[c * CH + h * H:c * CH + (h + 1) * H, :, :].rearrange(
                    "j b t -> b j t"
                )
                dst = ebf[h * BL:(h + 1) * BL, :].rearrange("b (j t) -> b j t", j=H)
                nc.gpsimd.dma_start(out=dst, in_=src)  # fp32 -> bf16 cast in DMA
            xtraw = xtr_pool.tile([T, CH * BL], BF16, tag="xtraw")
            for j in range(H):
                nc.sync.dma_start(
                    out=xtraw[:, j * 2 * BL:(j + 1) * 2 * BL],
                    in_=ebf[:, j * T:(j + 1) * T],
                    transpose=True,
                )
            x = pool.tile([T, CH * BL], BF16, tag=pool.name)
            nbat = (CH * BL + 511) // 512
            for k in range(nbat):
                sl = slice(k * 512, min((k + 1) * 512, CH * BL))
                nc.scalar.activation(out=x[:, sl], in_=xtraw[:, sl], func=Exp)
            x_tiles[c] = x
            xtraw_tiles[c] = xtraw

        def x_slice(i, raw=False):
            c, o = i // CH, i % CH
            t = (xtraw_tiles if raw else x_tiles)[c]
            col = o * 2 * BL if o < H else (o - H) * 2 * BL + BL
            return t[:, col:col + BL]

        nfwd_chunks = MID // CH + 1  # fwd consumes chunks 0 .. MID//CH (x_MID)
        for s in range(max(nfwd_chunks, nchunks - nfwd_chunks + 1)):
            cf, cb = s, nchunks - 1 - s
            if cf < nfwd_chunks:
                emit_chunk(cf, xf_pool)
            if cb >= nfwd_chunks and cb != cf:
                emit_chunk(cb, xb_pool)

        # ---------------- scan init ----------------
        # p_0 = exp(e_0 + start), w_{L-1} = exp(e_{L-1} + end)
        p_prev = p_pool.tile([T, BL], BF16, tag="p_f")
        nc.scalar.activation(
            out=p_prev[:], in_=x_slice(0, raw=True), func=Exp, bias=start_col[:]
        )
        w_prev = p_pool.tile([T, BL], BF16, tag="p_b")
        nc.scalar.activation(
            out=w_prev[:], in_=x_slice(L - 1, raw=True), func=Exp, bias=end_col[:]
        )

        def renorm(p_cur, c_row, tag):
            s_ps = ps_misc.tile([1, BL], FP32, space="PSUM", tag="s_ps")
            nc.tensor.matmul(
                out=s_ps[:], lhsT=ones_col_bf[:], rhs=p_cur[:], start=True, stop=True
            )
            rec32 = sm_pool.tile([1, BL], FP32, tag="rec32")
            nc.vector.reciprocal(out=rec32[:], in_=s_ps[:])
            recbf = sm_pool.tile([1, BL], BF16, tag="recbf")
            nc.vector.tensor_copy(out=recbf[:], in_=rec32[:])
            lnr = sm_pool.tile([1, BL], FP32, tag="lnr")
            nc.scalar.activation(out=lnr[:], in_=recbf[:], func=Ln)
            nc.vector.tensor_tensor(out=c_row[:], in0=c_row[:], in1=lnr[:], op=Sub)
            bc_ps = ps_misc.tile([128, BL], FP32, space="PSUM", tag="bc_ps")
            nc.tensor.matmul(
                out=bc_ps[:], lhsT=ones_row_bf[:], rhs=recbf[:], start=True, stop=True
            )
            p_new = p_pool.tile([T, BL], BF16, tag=tag)
            nc.vector.tensor_tensor(out=p_new[:], in0=bc_ps[:], in1=p_cur[:], op=Mult)
            return p_new

        # ---------------- interleaved forward/backward rounds ----------------
        # fwd round i (1..MID):    p_i = (E^T p_{i-1}) * x_i
        # bwd round j (L-1..MID+2): w_{j-1} = x_{j-1} * (E w_j)
        # final bwd matmul (j=MID+1) leaves v_MID = E w_{MID+1} in PSUM.
        nfwd = MID
        nbwd = L - 1 - MID  # matmul count; last one has no multiply
        v_mid_ps = None
        for r in range(max(nfwd, nbwd)):
            if r < nfwd:
                i = r + 1
                qf = ps_q.tile([T, BL], FP32, space="PSUM", tag="qf")
                nc.tensor.matmul(
                    out=qf[:], lhsT=E_bf[:], rhs=p_prev[:], start=True, stop=True
                )
                p_new = p_pool.tile([T, BL], BF16, tag="p_f")
                nc.vector.tensor_tensor(
                    out=p_new[:], in0=qf[:], in1=x_slice(i), op=Mult
                )
                p_prev = p_new
                if i % RENORM == 0 and i < nfwd:
                    p_prev = renorm(p_prev, c_f, "p_f")
            if r < nbwd:
                j = L - 1 - r
                qb = ps_q.tile([T, BL], FP32, space="PSUM", tag="qb")
                nc.tensor.matmul(
                    out=qb[:], lhsT=ET_bf[:], rhs=w_prev[:], start=True, stop=True
                )
                if j == MID + 1:
                    v_mid_ps = qb
                else:
                    w_new = p_pool.tile([T, BL], BF16, tag="p_b")
                    nc.vector.tensor_tensor(
                        out=w_new[:], in0=qb[:], in1=x_slice(j - 1), op=Mult
                    )
                    w_prev = w_new
                    if r % RENORM == RENORM // 2 and r < nbwd - 2:
                        w_prev = renorm(w_prev, c_b, "p_b")

        # ---------------- combine: logZ = ln(sum_t p_MID * v_MID) + c_f + c_b ----
        prod = sm_pool.tile([T, BL], FP32, tag="prod")
        nc.vector.tensor_tensor(
            out=prod[:], in0=v_mid_ps[:], in1=p_prev[:], op=Mult
        )
        zsum_ps = ps_misc.tile([1, BL], FP32, space="PSUM", tag="zsum")
        nc.tensor.matmul(
            out=zsum_ps[:], lhsT=ones_col_f32[:], rhs=prod[:], start=True, stop=True
        )
        z_row = sm_pool.tile([1, BL], FP32, tag="z_row")
        nc.scalar.activation(out=z_row[:], in_=zsum_ps[:], func=Ln)
        nc.vector.tensor_tensor(out=z_row[:], in0=z_row[:], in1=c_f[:], op=Add)
        nc.vector.tensor_tensor(out=z_row[:], in0=z_row[:], in1=c_b[:], op=Add)
        # compensate the 2^-7 folded into E: (L-1) matmuls total
        nc.vector.tensor_scalar(
            out=z_row[:], in0=z_row[:], scalar1=float((L - 1) * 7 * np.log(2.0)),
            scalar2=None, op0=Add,
        )
        nc.sync.dma_start(out=out_z[:], in_=z_row[:])

    return nc




def build_crf_kernel_v2(L=L, BL=BL, T=T, CH=32, S=16):
    """v2: segmented scan via rank-1 probe decomposition.

    Products of positive matrices contract projectively (Birkhoff): each
    step map D_x E^T shrinks Hilbert-metric diameter by ~tanh(0.1) (since
    |transitions| <= 0.1), so a 32-step segment map is rank-1 to ~1e-32.
    Each segment is evaluated independently with a forward probe
    u_s = M_s w and a backward probe rho_s = M_s^T z; the log-partition
    telescopes into per-segment scalars:

      Z_b = (rho_{S-1} . u_{S-2}) * prod_{s=1}^{S-2} (rho_s . u_{s-1}) / g_s
      g_s = sum_t u_s[t]

    with u_0 seeded exactly with p_0 = exp(start + e_0) and rho_{S-1}
    seeded with exp(end). This removes the 511-step serial latency chain:
    only n = L/S rounds of wide (T x (S-1)*BL) ops remain.
    """
    assert L % CH == 0 and CH % 2 == 0 and L % S == 0
    n = L // S
    nchunks = L // CH
    NP = S - 1            # probe slots per direction
    W = NP * BL           # probe tile width
    TCH = min(128, L)
    ntch = (L + TCH - 1) // TCH
    GW = BL

    nc = bass.Bass()

    emis = nc.declare_dram_parameter("emissions", [L, BL, T], FP32, isOutput=False)
    tags = nc.declare_dram_parameter("tags", [L, BL], I32, isOutput=False)
    start_t = nc.declare_dram_parameter("start_t", [T, 1], FP32, isOutput=False)
    end_t = nc.declare_dram_parameter("end_t", [T, 1], FP32, isOutput=False)
    trans = nc.declare_dram_parameter("trans", [T, T], FP32, isOutput=False)
    out_z = nc.declare_dram_parameter("out_z", [1, BL], FP32, isOutput=True)
    out_gold = nc.declare_dram_parameter("out_gold", [TCH, 1], FP32, isOutput=True)
    out_trans = nc.declare_dram_parameter("out_trans", [TCH, 1], FP32, isOutput=True)
    out_se = nc.declare_dram_parameter("out_se", [16, 8], FP32, isOutput=True)

    from contextlib import ExitStack

    with TileContext(nc) as tc, ExitStack() as es:
        cpool = es.enter_context(tc.tile_pool(name="consts", bufs=1))
        ebf_pool = es.enter_context(tc.tile_pool(name="ebf", bufs=3))
        xtr_pool = es.enter_context(tc.tile_pool(name="xtraw", bufs=3))
        sm_pool = es.enter_context(tc.tile_pool(name="small", bufs=2))
        num_pool = es.enter_context(tc.tile_pool(name="numer", bufs=1))
        tg_pool = es.enter_context(tc.tile_pool(name="tagt", bufs=2))
        ps_q = es.enter_context(tc.tile_pool(name="ps_q", bufs=1, space="PSUM"))
        ps_misc = es.enter_context(tc.tile_pool(name="ps_misc", bufs=1, space="PSUM"))

        # ---------------- constants ----------------
        trans_sb = cpool.tile([T, T], FP32, tag="trans_sb")
        nc.sync.dma_start(out=trans_sb[:], in_=trans[:])
        ident = cpool.tile([128, 128], FP32, tag="ident")
        make_identity(nc, ident[:])
        # Fold 2^-7 into E so per-step mass growth is ~1 (compensated by
        # +(L-1)*7*ln2 at the end); otherwise the 128-way sum overflows.
        LOG_SCALE = -7.0 * float(np.log(2.0))
        lsc_col = cpool.tile([128, 1], FP32, tag="lsc_col")
        nc.vector.memset(lsc_col[:], LOG_SCALE)
        E_bf = cpool.tile([T, T], BF16, tag="E_bf")
        nc.scalar.activation(out=E_bf[:], in_=trans_sb[:], func=Exp, bias=lsc_col[:])
        transT_ps = ps_misc.tile([T, T], FP32, space="PSUM", tag="misc")
        nc.tensor.transpose(out=transT_ps[:], in_=trans_sb[:], identity=ident[:])
        ET_bf = cpool.tile([T, T], BF16, tag="ET_bf")
        nc.scalar.activation(
            out=ET_bf[:], in_=transT_ps[:], func=Exp, bias=lsc_col[:]
        )
        start_col = cpool.tile([T, 1], FP32, tag="start_col")
        nc.sync.dma_start(out=start_col[:], in_=start_t[:])
        end_col = cpool.tile([T, 1], FP32, tag="end_col")
        nc.sync.dma_start(out=end_col[:], in_=end_t[:])
        ones_col_f32 = cpool.tile([128, 1], FP32, tag="ones_col_f32")
        nc.vector.memset(ones_col_f32[:], 1.0)
        ones_col_bf = cpool.tile([128, 1], BF16, tag="ones_col_bf")
        nc.vector.memset(ones_col_bf[:], 1.0)

        # ---------------- numerator (indirect gathers) ----------------
        gold_idx = num_pool.tile([TCH, L * BL // TCH], I32, tag="gold_idx")
        trans_idx = num_pool.tile([TCH, L * BL // TCH], I32, tag="trans_idx")
        for c in range(ntch):
            tcur = tg_pool.tile([TCH, BL], I32, tag="tags_cur")
            nc.sync.dma_start(out=tcur[:], in_=tags[c * TCH:(c + 1) * TCH, :])
            gsl = gold_idx[:, c * GW:(c + 1) * GW]
            nc.gpsimd.iota(
                gsl, pattern=[[T, BL]], base=c * TCH * BL * T,
                channel_multiplier=BL * T,
            )
            nc.vector.tensor_tensor(out=gsl, in0=gsl, in1=tcur[:], op=Add)
            tprev = tg_pool.tile([TCH, BL], I32, tag="tags_prev")
            if c == 0:
                nc.vector.memset(tprev[0:1, :], 0)
                nc.sync.dma_start(out=tprev[1:TCH, :], in_=tags[0:TCH - 1, :])
            else:
                nc.sync.dma_start(
                    out=tprev[:], in_=tags[c * TCH - 1:(c + 1) * TCH - 1, :]
                )
            tsl = trans_idx[:, c * GW:(c + 1) * GW]
            nc.vector.tensor_scalar(
                out=tsl, in0=tprev[:], scalar1=T, scalar2=None, op0=Mult
            )
            nc.vector.tensor_tensor(out=tsl, in0=tsl, in1=tcur[:], op=Add)
        nc.vector.memset(trans_idx[0:1, 0:GW], 1 << 24)

        gvals = num_pool.tile([TCH, L * BL // TCH], FP32, tag="gvals")
        nc.gpsimd.indirect_dma_start(
            out=gvals[:], out_offset=None, in_=emis[:],
            in_offset=bass.IndirectOffsetOnAxis(ap=gold_idx[:], axis=2),
            bounds_check=L * BL * T - 1, oob_is_err=False,
        )
        tvals = num_pool.tile([TCH, L * BL // TCH], FP32, tag="tvals")
        nc.gpsimd.indirect_dma_start(
            out=tvals[:], out_offset=None, in_=trans[:],
            in_offset=bass.IndirectOffsetOnAxis(ap=trans_idx[:], axis=1),
            bounds_check=T * T - 1, oob_is_err=False,
        )
        gold_red = num_pool.tile([TCH, 1], FP32, tag="gold_red")
        nc.vector.tensor_reduce(
            out=gold_red[:], in_=gvals[:], axis=mybir.AxisListType.X, op=Add
        )
        trans_red = num_pool.tile([TCH, 1], FP32, tag="trans_red")
        nc.vector.tensor_reduce(
            out=trans_red[:], in_=tvals[:], axis=mybir.AxisListType.X, op=Add
        )
        nc.sync.dma_start(out=out_gold[:], in_=gold_red[:])
        nc.sync.dma_start(out=out_trans[:], in_=trans_red[:])

        se_idx = num_pool.tile([16, 8], I32, tag="se_idx")
        nc.sync.dma_start(
            out=se_idx[:, 0:4], in_=tags[0:1, :].rearrange("o (p j) -> (o p) j", p=16)
        )
        nc.sync.dma_start(
            out=se_idx[:, 4:8],
            in_=tags[L - 1:L, :].rearrange("o (p j) -> (o p) j", p=16),
        )
        se_vals = num_pool.tile([16, 8], FP32, tag="se_vals")
        nc.gpsimd.indirect_dma_start(
            out=se_vals[:, 0:4], out_offset=None, in_=start_t[:],
            in_offset=bass.IndirectOffsetOnAxis(ap=se_idx[:, 0:4], axis=1),
            bounds_check=T - 1, oob_is_err=False,
        )
        nc.gpsimd.indirect_dma_start(
            out=se_vals[:, 4:8], out_offset=None, in_=end_t[:],
            in_offset=bass.IndirectOffsetOnAxis(ap=se_idx[:, 4:8], axis=1),
            bounds_check=T - 1, oob_is_err=False,
        )
        nc.sync.dma_start(out=out_se[:], in_=se_vals[:])

        # -------- emissions: cast + xbar transpose + exp into x_all --------
        H = CH // 2
        x_all = cpool.tile([T, L * BL], BF16, tag="x_all")  # step-major cols
        xtraw_tiles = {}

        def emit_chunk(c):
            ebf = ebf_pool.tile([2 * BL, H * T], BF16, tag="ebf")
            for h in range(2):
                src = emis[c * CH + h * H:c * CH + (h + 1) * H, :, :].rearrange(
                    "j b t -> b j t"
                )
                dst = ebf[h * BL:(h + 1) * BL, :].rearrange("b (j t) -> b j t", j=H)
                nc.gpsimd.dma_start(out=dst, in_=src)  # fp32->bf16 cast
            xtraw = xtr_pool.tile([T, CH * BL], BF16, tag="xtraw")
            for j in range(H):
                nc.sync.dma_start(
                    out=xtraw[:, j * 2 * BL:(j + 1) * 2 * BL],
                    in_=ebf[:, j * T:(j + 1) * T],
                    transpose=True,
                )
            # exp with strided out AP: pair j holds steps c*CH+j (first BL
            # cols) and c*CH+H+j; route each to x_all column step*BL.
            src_ap = xtraw[:].rearrange("p (j h b) -> p j h b", j=H, h=2)
            dst_ap = x_all[
                :, c * CH * BL:(c + 1) * CH * BL
            ].rearrange("p (h j b) -> p j h b", h=2, j=H)
            nc.scalar.activation(out=dst_ap, in_=src_ap, func=Exp)
            xtraw_tiles[c] = xtraw

        for c in range(nchunks):
            emit_chunk(c)

        # 4D step-indexed view of x_all: (p, segment, step-in-seg, b)
        x4 = x_all[:].rearrange("p (s r b) -> p s r b", s=S, b=BL)

        # ---------------- probe state + inits ----------------
        uw = cpool.tile([T, 2 * W], BF16, tag="uw")  # [u | w]
        # u slot s = segment s (0..S-2); w slot s-1 = segment s (1..S-1)
        nc.scalar.activation(
            out=uw[:, 0:BL], in_=xtraw_tiles[0][:, 0:BL], func=Exp,
            bias=start_col[:],
        )
        nc.vector.memset(uw[:, BL:W], 1.0)
        lastc = nchunks - 1
        lcol = (CH - 1 - H) * 2 * BL + BL  # raw e_{L-1} in xtraw[lastc]
        nc.scalar.activation(
            out=uw[:, W + (S - 2) * BL:2 * W],
            in_=xtraw_tiles[lastc][:, lcol:lcol + BL], func=Exp, bias=end_col[:],
        )
        # w slots 0..S-3 (segments 1..S-2) init = x at segment hi = s*n+n-1
        nc.vector.tensor_copy(
            out=uw[:, W:W + (S - 2) * BL].rearrange("p (s b) -> p s b", b=BL),
            in_=x4[:, 1:S - 1, n - 1, :],
        )

        # ---------------- probe rounds ----------------
        def mm_split(q_ap, lhsT, rhs_ap, wdt):
            # 512-col chunks: PSUM-bank-aligned (fp32 out) and <= matmul max N
            for m0 in range(0, wdt, 512):
                m1 = min(m0 + 512, wdt)
                nc.tensor.matmul(
                    out=q_ap[:, m0:m1], lhsT=lhsT[:], rhs=rhs_ap[:, m0:m1],
                    start=True, stop=True,
                )

        # fwd round 0: segments 1..S-2 only (segment 0 starts at step 1)
        q0 = ps_q.tile([T, (S - 2) * BL], FP32, space="PSUM", tag="q_f")
        mm_split(q0[:], E_bf, uw[:, BL:W], (S - 2) * BL)
        nc.vector.tensor_tensor(
            out=uw[:, BL:W].rearrange("p (s b) -> p s b", b=BL),
            in0=q0[:].rearrange("p (s b) -> p s b", b=BL),
            in1=x4[:, 1:S - 1, 0, :], op=Mult,
        )
        for r in range(1, n):
            # fwd: u_s <- (E^T u_s) * x[s*n + r],  s = 0..S-2
            qf = ps_q.tile([T, W], FP32, space="PSUM", tag="q_f")
            mm_split(qf[:], E_bf, uw[:, 0:W], W)
            nc.vector.tensor_tensor(
                out=uw[:, 0:W].rearrange("p (s b) -> p s b", b=BL),
                in0=qf[:].rearrange("p (s b) -> p s b", b=BL),
                in1=x4[:, 0:S - 1, r, :], op=Mult,
            )
            # bwd: w_s <- x[s*n + n-2-rb] * (E w_s),  s = 1..S-1, rb = r-1
            rb = r - 1
            qb = ps_q.tile([T, W], FP32, space="PSUM", tag="q_b")
            mm_split(qb[:], ET_bf, uw[:, W:2 * W], W)
            nc.vector.tensor_tensor(
                out=uw[:, W:2 * W].rearrange("p (s b) -> p s b", b=BL),
                in0=qb[:].rearrange("p (s b) -> p s b", b=BL),
                in1=x4[:, 1:S, n - 2 - rb, :], op=Mult,
            )
        # final bwd matmul: rho_s = E @ w_s (kept in PSUM for the combine)
        rho = ps_q.tile([T, W], FP32, space="PSUM", tag="q_b")
        mm_split(rho[:], ET_bf, uw[:, W:2 * W], W)

        # ---------------- combine ----------------
        # d_s = rho_s . u_{s-1} (slots aligned); g_s = sum_t u_s, s=1..S-2
        prod = sm_pool.tile([T, W], FP32, tag="prod")
        nc.vector.tensor_tensor(out=prod[:], in0=rho[:], in1=uw[:, 0:W], op=Mult)
        drow_ps = ps_misc.tile([1, W], FP32, space="PSUM", tag="misc")
        for m0 in range(0, W, 512):
            m1 = min(m0 + 512, W)
            nc.tensor.matmul(
                out=drow_ps[:, m0:m1], lhsT=ones_col_f32[:], rhs=prod[:, m0:m1],
                start=True, stop=True,
            )
        ln_d = sm_pool.tile([1, W], FP32, tag="ln_d")
        nc.scalar.activation(out=ln_d[:], in_=drow_ps[:], func=Ln)
        grow_ps = ps_misc.tile([1, (S - 2) * BL], FP32, space="PSUM", tag="misc")
        for m0 in range(0, (S - 2) * BL, 512):
            m1 = min(m0 + 512, (S - 2) * BL)
            nc.tensor.matmul(
                out=grow_ps[:, m0:m1], lhsT=ones_col_bf[:],
                rhs=uw[:, BL + m0:BL + m1], start=True, stop=True,
            )
        ln_g = sm_pool.tile([1, (S - 2) * BL], FP32, tag="ln_g")
        nc.scalar.activation(out=ln_g[:], in_=grow_ps[:], func=Ln)
        zred = sm_pool.tile([1, BL], FP32, tag="zred")
        nc.vector.tensor_reduce(
            out=zred[:], in_=ln_d[:].rearrange("p (s b) -> p b s", b=BL),
            axis=mybir.AxisListType.X, op=Add,
        )
        gred = sm_pool.tile([1, BL], FP32, tag="gred")
        nc.vector.tensor_reduce(
            out=gred[:], in_=ln_g[:].rearrange("p (s b) -> p b s", b=BL),
            axis=mybir.AxisListType.X, op=Add,
        )
        z_row = sm_pool.tile([1, BL], FP32, tag="z_row")
        nc.vector.tensor_tensor(out=z_row[:], in0=zred[:], in1=gred[:], op=Sub)
        nc.vector.tensor_scalar(
            out=z_row[:], in0=z_row[:], scalar1=float((L - 1) * 7 * np.log(2.0)),
            scalar2=None, op0=Add,
        )
        nc.sync.dma_start(out=out_z[:], in_=z_row[:])

    return nc


def _split_multi_waits(nc):
    """Workaround: this walrus encodes at most ONE sync-wait per instruction
    ("Too many sync wait commands"). Move extra waits onto same-engine NoOps
    inserted immediately before the instruction (engine blocks on each in
    program order, so semantics are identical)."""
    for fn in nc.m.functions:
        for bb in fn.blocks:
            insts = bb.instructions
            i = 0
            while i < len(insts):
                inst = insts[i]
                si = inst.sync_info
                if si is not None and si.on_wait and len(si.on_wait) > 1:
                    waits = list(si.on_wait)
                    for k, wsync in enumerate(waits[:-1]):
                        nop = mybir.InstNoOp(
                            name=f"{inst.name}-w{k}",
                            engine=inst.engine,
                            ins=[],
                            outs=[],
                            sync_info=mybir.SyncInfo(on_wait=[wsync], on_update=[]),
                        )
                        insts.insert(i, nop)
                        i += 1
                    inst.sync_info = mybir.SyncInfo(
                        on_wait=[waits[-1]], on_update=list(si.on_update or [])
                    )
                i += 1
    return nc


_NC_CACHE = {}


def _get_nc():
    key = "full"
    if key not in _NC_CACHE:
        builder = (
            build_crf_kernel_v2
            if int(os.environ.get("CRF_V2", "1"))
            else build_crf_kernel
        )
        _NC_CACHE[key] = _split_multi_waits(builder())
    return _NC_CACHE[key]


def make_in_maps(emissions, tags, start_transitions, end_transitions, transitions):
    emissions = np.ascontiguousarray(np.asarray(emissions, dtype=np.float32))
    tags = np.ascontiguousarray(np.asarray(tags).astype(np.int32))
    start = np.ascontiguousarray(
        np.asarray(start_transitions, dtype=np.float32).reshape(T, 1)
    )
    end = np.ascontiguousarray(
        np.asarray(end_transitions, dtype=np.float32).reshape(T, 1)
    )
    trans = np.ascontiguousarray(np.asarray(transitions, dtype=np.float32))
    in_maps = []
    for i in range(NCORES):
        sl = slice(i * BL, (i + 1) * BL)
        in_maps.append({
            "emissions": np.ascontiguousarray(emissions[:, sl, :]),
            "tags": np.ascontiguousarray(tags[:, sl]),
            "start_t": start,
            "end_t": end,
            "trans": trans,
        })
    return in_maps


def combine_outputs(results):
    log_den = 0.0
    log_num = 0.0
    for res in results:
        log_den += np.asarray(res["out_z"], dtype=np.float64).sum()
        log_num += np.asarray(res["out_gold"], dtype=np.float64).sum()
        log_num += np.asarray(res["out_trans"], dtype=np.float64).sum()
        log_num += np.asarray(res["out_se"], dtype=np.float64).sum()
    return np.float32((log_den - log_num) / B)


def kernel(emissions, tags, mask, start_transitions, end_transitions, transitions):
    mask = np.asarray(mask)
    assert mask.all(), "kernel assumes mask of all ones (spec fill=ones)"
    from concourse.bass_utils import run_bass_kernel_spmd

    nc = _get_nc()
    in_maps = make_in_maps(
        emissions, tags, start_transitions, end_transitions, transitions
    )
    trace = bool(int(os.environ.get("CRF_TRACE", "0")))
    if trace:
        try:
            import importlib.util as _iu

            try:
                from antenv import axon_hooks as _hooks
            except ImportError:
                # antenv may already be cached from a copy lacking axon_hooks;
                # load ours by path and graft it into the package.
                import antenv

                _spec = _iu.spec_from_file_location(
                    "antenv.axon_hooks", "/opt/trn_rl_repo/antenv/axon_hooks.py"
                )
                _hooks = _iu.module_from_spec(_spec)
                _spec.loader.exec_module(_hooks)
                sys.modules["antenv.axon_hooks"] = _hooks
                antenv.axon_hooks = _hooks

            if _hooks.get_axon_ntff_profile_hook() is None:
                from trn_agent_boot.trn_boot import _ntff_profile_via_ctypes

                _hooks.set_axon_ntff_profile_hook(
                    _ntff_profile_via_ctypes("/opt/axon/libaxon_pjrt.so")
                )
        except Exception as e:  # profiling is best-effort
            print(f"NTFF hook install failed ({e}); running untraced")
            trace = False
    br = run_bass_kernel_spmd(nc, in_maps, list(range(NCORES)), trace=trace)
    if trace and br.exec_time_ns is not None:
        print(f"HW exec time: {br.exec_time_ns} ns")
        kernel.last_exec_time_ns = br.exec_time_ns
    return combine_outputs(br.results)


kernel.last_exec_time_ns = None


# revision 25
# speedup vs baseline: 2.6994x; 2.6994x over previous
"""CRF loss (negative log-likelihood) kernel for Trainium2, 8 NeuronCores.

Strategy (data-parallel over batch, per the sharding hint):
  - Each of 8 cores gets B/8 = 64 sequences; the same NEFF runs SPMD on all
    cores with per-core input shards, and the host sums the tiny partials.
  - Denominator (log partition, the heavy part): the forward recursion
    p_i = diag(x_i) E^T p_{i-1} (x = exp(emissions), E = exp(transitions))
    is a product of positive matrices, which contracts projectively
    (Birkhoff) by ~tanh(0.1) per step since |transitions| <= 0.1. A 32-step
    segment map is therefore numerically rank-1, so the 511-step serial
    chain splits into 16 independent segments evaluated with forward
    probes u_s = M_s w (full length, carries the scale) and backward
    probes rho_s ~ M_s^T z (16 steps suffice), recombined exactly via
      Z_b = (rho_{S-1}.u_{S-2}) * prod_s (rho_s.u_{s-1}) / (rho_s.w).
    Segments run as wide (128 x 960) matmul+multiply rounds — latency
    chains are 32 long instead of 511. A 2^-7 scale folded into E keeps
    the exp-domain values in range (compensated by +511*7*ln2).
  - Emissions stream: SWDGE DMA casts fp32->bf16 in a (step, b*t) layout
    (32KB contiguous per partition), batched ACT exp, then one xbar
    transpose-DMA per 128-step chunk (3D out AP) into x[t, b*L+i].
    Probes run in two segment groups so the scan overlaps the stream.
  - Numerator (gold path score) via indirect DMA element gathers:
    emissions at gold tags, transitions at tag pairs, start/end; reduced
    on device. bf16 is safe for the denominator because the loss gradient
    w.r.t. emissions is bounded (errors average out); the numerator reads
    raw fp32 values.
"""

import os
import sys

import numpy as np

for _p in ("/opt/trn_rl_repo", os.path.expanduser("~/.axon_site/_ro/trn_rl_repo")):
    if os.path.isdir(_p):
        if _p not in sys.path:
            sys.path.insert(0, _p)
        break

import concourse.bass as bass  # noqa: E402
from concourse import mybir  # noqa: E402
from concourse.masks import make_identity  # noqa: E402
from concourse.tile import TileContext  # noqa: E402

FP32 = mybir.dt.float32
BF16 = mybir.dt.bfloat16
I32 = mybir.dt.int32
Exp = mybir.ActivationFunctionType.Exp
Ln = mybir.ActivationFunctionType.Ln
Add = mybir.AluOpType.add
Sub = mybir.AluOpType.subtract
Mult = mybir.AluOpType.mult

L, B, T = 512, 512, 128
NCORES = 8
BL = B // NCORES  # 64 sequences per core


def build_crf_kernel(L=L, BL=BL, T=T, CH=32, RENORM=64):
    """Build the per-core Bass kernel (SPMD: same NEFF, different inputs)."""
    assert L % CH == 0 and CH % 2 == 0
    nchunks = L // CH
    MID = L // 2  # fwd covers steps 1..MID, bwd covers MID+1..L-1
    TCH = min(128, L)  # tags chunk (steps on partitions)
    ntch = (L + TCH - 1) // TCH
    GW = BL  # free width contributed per tags chunk in the gather tiles

    nc = bass.Bass()

    emis = nc.declare_dram_parameter("emissions", [L, BL, T], FP32, isOutput=False)
    tags = nc.declare_dram_parameter("tags", [L, BL], I32, isOutput=False)
    start_t = nc.declare_dram_parameter("start_t", [T, 1], FP32, isOutput=False)
    end_t = nc.declare_dram_parameter("end_t", [T, 1], FP32, isOutput=False)
    trans = nc.declare_dram_parameter("trans", [T, T], FP32, isOutput=False)
    out_z = nc.declare_dram_parameter("out_z", [1, BL], FP32, isOutput=True)
    out_gold = nc.declare_dram_parameter("out_gold", [TCH, 1], FP32, isOutput=True)
    out_trans = nc.declare_dram_parameter("out_trans", [TCH, 1], FP32, isOutput=True)
    out_se = nc.declare_dram_parameter("out_se", [16, 8], FP32, isOutput=True)

    from contextlib import ExitStack

    with TileContext(nc) as tc, ExitStack() as es:
        cpool = es.enter_context(tc.tile_pool(name="consts", bufs=1))
        ebf_pool = es.enter_context(tc.tile_pool(name="ebf", bufs=2))
        xtr_pool = es.enter_context(tc.tile_pool(name="xtraw", bufs=2))
        xf_pool = es.enter_context(tc.tile_pool(name="x_f", bufs=3))
        xb_pool = es.enter_context(tc.tile_pool(name="x_b", bufs=3))
        p_pool = es.enter_context(tc.tile_pool(name="pp", bufs=4))
        sm_pool = es.enter_context(tc.tile_pool(name="small", bufs=2))
        num_pool = es.enter_context(tc.tile_pool(name="numer", bufs=1))
        tg_pool = es.enter_context(tc.tile_pool(name="tagt", bufs=2))
        ps_q = es.enter_context(tc.tile_pool(name="ps_q", bufs=2, space="PSUM"))
        ps_misc = es.enter_context(tc.tile_pool(name="ps_misc", bufs=1, space="PSUM"))

        # ---------------- constants ----------------
        trans_sb = cpool.tile([T, T], FP32, tag="trans_sb")
        nc.sync.dma_start(out=trans_sb[:], in_=trans[:])
        ident = cpool.tile([128, 128], FP32, tag="ident")
        make_identity(nc, ident[:])
        # Fold a 2^-7 scale into E so per-step mass growth is ~1 (the
        # sum over 128 source tags would otherwise overflow in ~16 steps).
        # Compensated exactly by +(L-1)*7*ln2 on the final log-partition.
        LOG_SCALE = -7.0 * float(np.log(2.0))
        lsc_col = cpool.tile([128, 1], FP32, tag="lsc_col")
        nc.vector.memset(lsc_col[:], LOG_SCALE)
        E_bf = cpool.tile([T, T], BF16, tag="E_bf")
        nc.scalar.activation(out=E_bf[:], in_=trans_sb[:], func=Exp, bias=lsc_col[:])
        transT_ps = ps_misc.tile([T, T], FP32, space="PSUM", tag="transT")
        nc.tensor.transpose(out=transT_ps[:], in_=trans_sb[:], identity=ident[:])
        ET_bf = cpool.tile([T, T], BF16, tag="ET_bf")
        nc.scalar.activation(
            out=ET_bf[:], in_=transT_ps[:], func=Exp, bias=lsc_col[:]
        )

        start_col = cpool.tile([T, 1], FP32, tag="start_col")
        nc.sync.dma_start(out=start_col[:], in_=start_t[:])
        end_col = cpool.tile([T, 1], FP32, tag="end_col")
        nc.sync.dma_start(out=end_col[:], in_=end_t[:])

        ones_col_bf = cpool.tile([128, 1], BF16, tag="ones_col_bf")
        nc.vector.memset(ones_col_bf[:], 1.0)
        ones_col_f32 = cpool.tile([128, 1], FP32, tag="ones_col_f32")
        nc.vector.memset(ones_col_f32[:], 1.0)
        ones_row_bf = cpool.tile([1, 128], BF16, tag="ones_row_bf")
        nc.vector.memset(ones_row_bf[:], 1.0)
        ones_bl_bf = cpool.tile([128, BL], BF16, tag="ones_bl_bf")
        nc.vector.memset(ones_bl_bf[:], 1.0)

        c_f = sm_pool.tile([1, BL], FP32, tag="c_f")
        nc.vector.memset(c_f[:], 0.0)
        c_b = sm_pool.tile([1, BL], FP32, tag="c_b")
        nc.vector.memset(c_b[:], 0.0)

        # ---------------- numerator: gathers ----------------
        gold_idx = num_pool.tile([TCH, L * BL // TCH], I32, tag="gold_idx")
        trans_idx = num_pool.tile([TCH, L * BL // TCH], I32, tag="trans_idx")
        tags_cur = {}
        for c in range(ntch):
            tcur = tg_pool.tile([TCH, BL], I32, tag="tags_cur")
            nc.sync.dma_start(out=tcur[:], in_=tags[c * TCH:(c + 1) * TCH, :])
            tags_cur[c] = tcur
            gsl = gold_idx[:, c * GW:(c + 1) * GW]
            # gold flat index = (i*BL + b)*T + tags[i, b]
            nc.gpsimd.iota(
                gsl, pattern=[[T, BL]], base=c * TCH * BL * T,
                channel_multiplier=BL * T,
            )
            nc.vector.tensor_tensor(out=gsl, in0=gsl, in1=tcur[:], op=Add)

            tprev = tg_pool.tile([TCH, BL], I32, tag="tags_prev")
            if c == 0:
                nc.vector.memset(tprev[0:1, :], 0)
                nc.sync.dma_start(out=tprev[1:TCH, :], in_=tags[0:TCH - 1, :])
            else:
                nc.sync.dma_start(
                    out=tprev[:], in_=tags[c * TCH - 1:(c + 1) * TCH - 1, :]
                )
            tsl = trans_idx[:, c * GW:(c + 1) * GW]
            # trans flat index = tags[i-1]*T + tags[i]
            nc.vector.tensor_scalar(
                out=tsl, in0=tprev[:], scalar1=T, scalar2=None, op0=Mult
            )
            nc.vector.tensor_tensor(out=tsl, in0=tsl, in1=tcur[:], op=Add)
        # pair step 0 does not exist: poison its indices; bounds_check skips them
        nc.vector.memset(trans_idx[0:1, 0:GW], 1 << 24)

        gvals = num_pool.tile([TCH, L * BL // TCH], FP32, tag="gvals")
        nc.vector.memset(gvals[:], 0.0)  # OOB-skipped entries leave SBUF as-is
        nc.gpsimd.indirect_dma_start(
            out=gvals[:], out_offset=None, in_=emis[:],
            in_offset=bass.IndirectOffsetOnAxis(ap=gold_idx[:], axis=2),
            bounds_check=L * BL * T - 1, oob_is_err=False,
        )
        tvals = num_pool.tile([TCH, L * BL // TCH], FP32, tag="tvals")
        nc.vector.memset(tvals[:], 0.0)  # OOB-skipped entries leave SBUF as-is
        nc.gpsimd.indirect_dma_start(
            out=tvals[:], out_offset=None, in_=trans[:],
            in_offset=bass.IndirectOffsetOnAxis(ap=trans_idx[:], axis=1),
            bounds_check=T * T - 1, oob_is_err=False,
        )
        gold_red = num_pool.tile([TCH, 1], FP32, tag="gold_red")
        nc.vector.tensor_reduce(
            out=gold_red[:], in_=gvals[:], axis=mybir.AxisListType.X, op=Add
        )
        trans_red = num_pool.tile([TCH, 1], FP32, tag="trans_red")
        nc.vector.tensor_reduce(
            out=trans_red[:], in_=tvals[:], axis=mybir.AxisListType.X, op=Add
        )
        nc.sync.dma_start(out=out_gold[:], in_=gold_red[:])
        nc.sync.dma_start(out=out_trans[:], in_=trans_red[:])

        # start/end transition gathers (64 each)
        se_idx = num_pool.tile([16, 8], I32, tag="se_idx")
        nc.sync.dma_start(
            out=se_idx[:, 0:4], in_=tags[0:1, :].rearrange("o (p j) -> (o p) j", p=16)
        )
        nc.sync.dma_start(
            out=se_idx[:, 4:8],
            in_=tags[L - 1:L, :].rearrange("o (p j) -> (o p) j", p=16),
        )
        se_vals = num_pool.tile([16, 8], FP32, tag="se_vals")
        nc.gpsimd.indirect_dma_start(
            out=se_vals[:, 0:4], out_offset=None, in_=start_t[:],
            in_offset=bass.IndirectOffsetOnAxis(ap=se_idx[:, 0:4], axis=1),
            bounds_check=T - 1, oob_is_err=False,
        )
        nc.gpsimd.indirect_dma_start(
            out=se_vals[:, 4:8], out_offset=None, in_=end_t[:],
            in_offset=bass.IndirectOffsetOnAxis(ap=se_idx[:, 4:8], axis=1),
            bounds_check=T - 1, oob_is_err=False,
        )
        nc.sync.dma_start(out=out_se[:], in_=se_vals[:])

        # ---------------- emissions stream: cast + transpose + exp ----------------
        H = CH // 2
        x_tiles = {}      # chunk -> x tile (exp'ed, (t, b) layout)
        xtraw_tiles = {}  # chunk -> pre-exp transposed tile (for biased inits)

        def emit_chunk(c, pool):
            ebf = ebf_pool.tile([2 * BL, H * T], BF16, tag="ebf")
            for h in range(2):
                src = emis[c * CH + h * H:c * CH + (h + 1) * H, :, :].rearrange(
                    "j b t -> b j t"
                )
                dst = ebf[h * BL:(h + 1) * BL, :].rearrange("b (j t) -> b j t", j=H)
                nc.gpsimd.dma_start(out=dst, in_=src)  # fp32 -> bf16 cast in DMA
            xtraw = xtr_pool.tile([T, CH * BL], BF16, tag="xtraw")
            for j in range(H):
                nc.sync.dma_start(
                    out=xtraw[:, j * 2 * BL:(j + 1) * 2 * BL],
                    in_=ebf[:, j * T:(j + 1) * T],
                    transpose=True,
                )
            x = pool.tile([T, CH * BL], BF16, tag=pool.name)
            nbat = (CH * BL + 511) // 512
            for k in range(nbat):
                sl = slice(k * 512, min((k + 1) * 512, CH * BL))
                nc.scalar.activation(out=x[:, sl], in_=xtraw[:, sl], func=Exp)
            x_tiles[c] = x
            xtraw_tiles[c] = xtraw

        def x_slice(i, raw=False):
            c, o = i // CH, i % CH
            t = (xtraw_tiles if raw else x_tiles)[c]
            col = o * 2 * BL if o < H else (o - H) * 2 * BL + BL
            return t[:, col:col + BL]

        nfwd_chunks = MID // CH + 1  # fwd consumes chunks 0 .. MID//CH (x_MID)
        for s in range(max(nfwd_chunks, nchunks - nfwd_chunks + 1)):
            cf, cb = s, nchunks - 1 - s
            if cf < nfwd_chunks:
                emit_chunk(cf, xf_pool)
            if cb >= nfwd_chunks and cb != cf:
                emit_chunk(cb, xb_pool)

        # ---------------- scan init ----------------
        # p_0 = exp(e_0 + start), w_{L-1} = exp(e_{L-1} + end)
        p_prev = p_pool.tile([T, BL], BF16, tag="p_f")
        nc.scalar.activation(
            out=p_prev[:], in_=x_slice(0, raw=True), func=Exp, bias=start_col[:]
        )
        w_prev = p_pool.tile([T, BL], BF16, tag="p_b")
        nc.scalar.activation(
            out=w_prev[:], in_=x_slice(L - 1, raw=True), func=Exp, bias=end_col[:]
        )

        def renorm(p_cur, c_row, tag):
            s_ps = ps_misc.tile([1, BL], FP32, space="PSUM", tag="s_ps")
            nc.tensor.matmul(
                out=s_ps[:], lhsT=ones_col_bf[:], rhs=p_cur[:], start=True, stop=True
            )
            rec32 = sm_pool.tile([1, BL], FP32, tag="rec32")
            nc.vector.reciprocal(out=rec32[:], in_=s_ps[:])
            recbf = sm_pool.tile([1, BL], BF16, tag="recbf")
            nc.vector.tensor_copy(out=recbf[:], in_=rec32[:])
            lnr = sm_pool.tile([1, BL], FP32, tag="lnr")
            nc.scalar.activation(out=lnr[:], in_=recbf[:], func=Ln)
            nc.vector.tensor_tensor(out=c_row[:], in0=c_row[:], in1=lnr[:], op=Sub)
            bc_ps = ps_misc.tile([128, BL], FP32, space="PSUM", tag="bc_ps")
            nc.tensor.matmul(
                out=bc_ps[:], lhsT=ones_row_bf[:], rhs=recbf[:], start=True, stop=True
            )
            p_new = p_pool.tile([T, BL], BF16, tag=tag)
            nc.vector.tensor_tensor(out=p_new[:], in0=bc_ps[:], in1=p_cur[:], op=Mult)
            return p_new

        # ---------------- interleaved forward/backward rounds ----------------
        # fwd round i (1..MID):    p_i = (E^T p_{i-1}) * x_i
        # bwd round j (L-1..MID+2): w_{j-1} = x_{j-1} * (E w_j)
        # final bwd matmul (j=MID+1) leaves v_MID = E w_{MID+1} in PSUM.
        nfwd = MID
        nbwd = L - 1 - MID  # matmul count; last one has no multiply
        v_mid_ps = None
        for r in range(max(nfwd, nbwd)):
            if r < nfwd:
                i = r + 1
                qf = ps_q.tile([T, BL], FP32, space="PSUM", tag="qf")
                nc.tensor.matmul(
                    out=qf[:], lhsT=E_bf[:], rhs=p_prev[:], start=True, stop=True
                )
                p_new = p_pool.tile([T, BL], BF16, tag="p_f")
                nc.vector.tensor_tensor(
                    out=p_new[:], in0=qf[:], in1=x_slice(i), op=Mult
                )
                p_prev = p_new
                if i % RENORM == 0 and i < nfwd:
                    p_prev = renorm(p_prev, c_f, "p_f")
            if r < nbwd:
                j = L - 1 - r
                qb = ps_q.tile([T, BL], FP32, space="PSUM", tag="qb")
                nc.tensor.matmul(
                    out=qb[:], lhsT=ET_bf[:], rhs=w_prev[:], start=True, stop=True
                )
                if j == MID + 1:
                    v_mid_ps = qb
                else:
                    w_new = p_pool.tile([T, BL], BF16, tag="p_b")
                    nc.vector.tensor_tensor(
                        out=w_new[:], in0=qb[:], in1=x_slice(j - 1), op=Mult
                    )
                    w_prev = w_new
                    if r % RENORM == RENORM // 2 and r < nbwd - 2:
                        w_prev = renorm(w_prev, c_b, "p_b")

        # ---------------- combine: logZ = ln(sum_t p_MID * v_MID) + c_f + c_b ----
        prod = sm_pool.tile([T, BL], FP32, tag="prod")
        nc.vector.tensor_tensor(
            out=prod[:], in0=v_mid_ps[:], in1=p_prev[:], op=Mult
        )
        zsum_ps = ps_misc.tile([1, BL], FP32, space="PSUM", tag="zsum")
        nc.tensor.matmul(
            out=zsum_ps[:], lhsT=ones_col_f32[:], rhs=prod[:], start=True, stop=True
        )
        z_row = sm_pool.tile([1, BL], FP32, tag="z_row")
        nc.scalar.activation(out=z_row[:], in_=zsum_ps[:], func=Ln)
        nc.vector.tensor_tensor(out=z_row[:], in0=z_row[:], in1=c_f[:], op=Add)
        nc.vector.tensor_tensor(out=z_row[:], in0=z_row[:], in1=c_b[:], op=Add)
        # compensate the 2^-7 folded into E: (L-1) matmuls total
        nc.vector.tensor_scalar(
            out=z_row[:], in0=z_row[:], scalar1=float((L - 1) * 7 * np.log(2.0)),
            scalar2=None, op0=Add,
        )
        nc.sync.dma_start(out=out_z[:], in_=z_row[:])

    # Postamble: drain + clear semaphores so the NEFF is re-executable
    # (without target_bir_lowering there is no preamble sem_clear).
    nc.reset()
    return nc




def build_crf_kernel_v2(L=L, BL=BL, T=T, S=16):
    """v2/v3: segmented scan via rank-1 probe decomposition.

    Products of positive matrices contract projectively (Birkhoff): each
    step map D_x E^T shrinks Hilbert-metric diameter by ~tanh(0.1) (since
    |transitions| <= 0.1), so a 32-step segment map is rank-1 to ~1e-32.
    Each segment is evaluated independently with a forward probe
    u_s = M_s w and a backward probe rho_s = M_s^T z; the log-partition
    telescopes into per-segment scalars:

      Z_b = (rho_{S-1} . u_{S-2}) * prod_{s=1}^{S-2} (rho_s . u_{s-1}) / g_s
      g_s = sum_t u_s[t]

    with u_0 seeded exactly with p_0 = exp(start + e_0) and rho_{S-1}
    seeded with exp(end). This removes the 511-step serial latency chain:
    only n = L/S rounds of wide ops remain. Probes run in two segment
    groups so the second half of the emissions stream overlaps the first
    group's scan.

    Emissions stream: SWDGE cast-DMA in (step, b*t) layout (32KB
    contiguous per partition), batched ACT exp, then ONE xbar
    transpose-DMA per 128-step chunk using a 3D out AP (out[t,b,i] =
    in[i, b*T+t]) into x_store[t, b*L + i].
    """
    assert L % S == 0
    n = L // S
    CH = 128                     # steps per emissions chunk (partition dim)
    nchunks = L // CH
    segs_per_chunk = CH // n
    NP = S - 1
    W = NP * BL
    TCH = min(128, L)
    ntch = (L + TCH - 1) // TCH
    GW = BL

    nc = bass.Bass()

    emis = nc.declare_dram_parameter("emissions", [L, BL, T], FP32, isOutput=False)
    tags = nc.declare_dram_parameter("tags", [L, BL], I32, isOutput=False)
    start_t = nc.declare_dram_parameter("start_t", [T, 1], FP32, isOutput=False)
    end_t = nc.declare_dram_parameter("end_t", [T, 1], FP32, isOutput=False)
    trans = nc.declare_dram_parameter("trans", [T, T], FP32, isOutput=False)
    out_z = nc.declare_dram_parameter("out_z", [1, BL], FP32, isOutput=True)
    out_gold = nc.declare_dram_parameter("out_gold", [TCH, 1], FP32, isOutput=True)
    out_trans = nc.declare_dram_parameter("out_trans", [TCH, 1], FP32, isOutput=True)
    out_se = nc.declare_dram_parameter("out_se", [16, 8], FP32, isOutput=True)

    from contextlib import ExitStack

    with TileContext(nc) as tc, ExitStack() as es:
        cpool = es.enter_context(tc.tile_pool(name="consts", bufs=1))
        ebf_pool = es.enter_context(tc.tile_pool(name="ebf", bufs=2))
        xe_pool = es.enter_context(tc.tile_pool(name="xebf", bufs=2))
        sm_pool = es.enter_context(tc.tile_pool(name="small", bufs=2))
        num_pool = es.enter_context(tc.tile_pool(name="numer", bufs=1))
        tg_pool = es.enter_context(tc.tile_pool(name="tagt", bufs=2))
        ps_q = es.enter_context(tc.tile_pool(name="ps_q", bufs=1, space="PSUM"))
        ps_misc = es.enter_context(tc.tile_pool(name="ps_misc", bufs=1, space="PSUM"))

        # ---------------- constants ----------------
        trans_sb = cpool.tile([T, T], FP32, tag="trans_sb")
        nc.sync.dma_start(out=trans_sb[:], in_=trans[:])
        ident = cpool.tile([128, 128], FP32, tag="ident")
        make_identity(nc, ident[:])
        # Fold 2^-7 into E so per-step mass growth is ~1 (compensated by
        # +(L-1)*7*ln2 at the end); otherwise the 128-way sum overflows.
        LOG_SCALE = -7.0 * float(np.log(2.0))
        lsc_col = cpool.tile([128, 1], FP32, tag="lsc_col")
        nc.vector.memset(lsc_col[:], LOG_SCALE)
        E_bf = cpool.tile([T, T], BF16, tag="E_bf")
        nc.scalar.activation(out=E_bf[:], in_=trans_sb[:], func=Exp, bias=lsc_col[:])
        transT_ps = ps_misc.tile([T, T], FP32, space="PSUM", tag="misc")
        nc.tensor.transpose(out=transT_ps[:], in_=trans_sb[:], identity=ident[:])
        ET_bf = cpool.tile([T, T], BF16, tag="ET_bf")
        nc.scalar.activation(
            out=ET_bf[:], in_=transT_ps[:], func=Exp, bias=lsc_col[:]
        )
        start_col = cpool.tile([T, 1], FP32, tag="start_col")
        nc.sync.dma_start(out=start_col[:], in_=start_t[:])
        end_col = cpool.tile([T, 1], FP32, tag="end_col")
        nc.sync.dma_start(out=end_col[:], in_=end_t[:])
        expstart_col = cpool.tile([T, 1], FP32, tag="expstart_col")
        nc.scalar.activation(out=expstart_col[:], in_=start_col[:], func=Exp)
        expend_col = cpool.tile([T, 1], FP32, tag="expend_col")
        nc.scalar.activation(out=expend_col[:], in_=end_col[:], func=Exp)
        ones_col_f32 = cpool.tile([128, 1], FP32, tag="ones_col_f32")
        nc.vector.memset(ones_col_f32[:], 1.0)
        ones_col_bf = cpool.tile([128, 1], BF16, tag="ones_col_bf")
        nc.vector.memset(ones_col_bf[:], 1.0)

        # ---------------- numerator (indirect gathers) ----------------
        gold_idx = num_pool.tile([TCH, L * BL // TCH], I32, tag="gold_idx")
        trans_idx = num_pool.tile([TCH, L * BL // TCH], I32, tag="trans_idx")
        for c in range(ntch):
            tcur = tg_pool.tile([TCH, BL], I32, tag="tags_cur")
            nc.sync.dma_start(out=tcur[:], in_=tags[c * TCH:(c + 1) * TCH, :])
            gsl = gold_idx[:, c * GW:(c + 1) * GW]
            nc.gpsimd.iota(
                gsl, pattern=[[T, BL]], base=c * TCH * BL * T,
                channel_multiplier=BL * T,
            )
            nc.vector.tensor_tensor(out=gsl, in0=gsl, in1=tcur[:], op=Add)
            tprev = tg_pool.tile([TCH, BL], I32, tag="tags_prev")
            if c == 0:
                nc.vector.memset(tprev[0:1, :], 0)
                nc.sync.dma_start(out=tprev[1:TCH, :], in_=tags[0:TCH - 1, :])
            else:
                nc.sync.dma_start(
                    out=tprev[:], in_=tags[c * TCH - 1:(c + 1) * TCH - 1, :]
                )
            tsl = trans_idx[:, c * GW:(c + 1) * GW]
            nc.vector.tensor_scalar(
                out=tsl, in0=tprev[:], scalar1=T, scalar2=None, op0=Mult
            )
            nc.vector.tensor_tensor(out=tsl, in0=tsl, in1=tcur[:], op=Add)
        nc.vector.memset(trans_idx[0:1, 0:GW], 1 << 24)

        gvals = num_pool.tile([TCH, L * BL // TCH], FP32, tag="gvals")
        nc.vector.memset(gvals[:], 0.0)  # OOB-skipped entries leave SBUF as-is
        nc.gpsimd.indirect_dma_start(
            out=gvals[:], out_offset=None, in_=emis[:],
            in_offset=bass.IndirectOffsetOnAxis(ap=gold_idx[:], axis=2),
            bounds_check=L * BL * T - 1, oob_is_err=False,
        )
        tvals = num_pool.tile([TCH, L * BL // TCH], FP32, tag="tvals")
        nc.vector.memset(tvals[:], 0.0)  # OOB-skipped entries leave SBUF as-is
        nc.gpsimd.indirect_dma_start(
            out=tvals[:], out_offset=None, in_=trans[:],
            in_offset=bass.IndirectOffsetOnAxis(ap=trans_idx[:], axis=1),
            bounds_check=T * T - 1, oob_is_err=False,
        )
        gold_red = num_pool.tile([TCH, 1], FP32, tag="gold_red")
        nc.vector.tensor_reduce(
            out=gold_red[:], in_=gvals[:], axis=mybir.AxisListType.X, op=Add
        )
        trans_red = num_pool.tile([TCH, 1], FP32, tag="trans_red")
        nc.vector.tensor_reduce(
            out=trans_red[:], in_=tvals[:], axis=mybir.AxisListType.X, op=Add
        )
        nc.sync.dma_start(out=out_gold[:], in_=gold_red[:])
        nc.sync.dma_start(out=out_trans[:], in_=trans_red[:])

        se_idx = num_pool.tile([16, 8], I32, tag="se_idx")
        nc.sync.dma_start(
            out=se_idx[:, 0:4], in_=tags[0:1, :].rearrange("o (p j) -> (o p) j", p=16)
        )
        nc.sync.dma_start(
            out=se_idx[:, 4:8],
            in_=tags[L - 1:L, :].rearrange("o (p j) -> (o p) j", p=16),
        )
        se_vals = num_pool.tile([16, 8], FP32, tag="se_vals")
        nc.gpsimd.indirect_dma_start(
            out=se_vals[:, 0:4], out_offset=None, in_=start_t[:],
            in_offset=bass.IndirectOffsetOnAxis(ap=se_idx[:, 0:4], axis=1),
            bounds_check=T - 1, oob_is_err=False,
        )
        nc.gpsimd.indirect_dma_start(
            out=se_vals[:, 4:8], out_offset=None, in_=end_t[:],
            in_offset=bass.IndirectOffsetOnAxis(ap=se_idx[:, 4:8], axis=1),
            bounds_check=T - 1, oob_is_err=False,
        )
        nc.sync.dma_start(out=out_se[:], in_=se_vals[:])

        # -------- emissions: cast + exp + one batched xbar transpose/chunk ------
        x_store = cpool.tile([T, BL * L], BF16, tag="x_store")  # [t, b*L + i]

        def emit_chunk(c):
            ebf = ebf_pool.tile([CH, BL * T], BF16, tag="ebf")
            nc.gpsimd.dma_start(
                out=ebf[:],
                in_=emis[c * CH:(c + 1) * CH, :, :].rearrange("i b t -> i (b t)"),
            )  # fp32->bf16 cast; 32KB contiguous per partition
            xebf = xe_pool.tile([CH, BL * T], BF16, tag="xebf")
            for k in range(4):
                sl = slice(k * BL * T // 4, (k + 1) * BL * T // 4)
                nc.scalar.activation(out=xebf[:, sl], in_=ebf[:, sl], func=Exp)
            # out[t, b, i] = in[i, b*T + t] for this chunk's i-range;
            # split by b-half across both HWDGE rings (SP + ACT) to halve
            # the serial xbar time.
            for h in range(2):
                dst = x_store[:].rearrange("p (b l) -> p b l", l=L)[
                    :, h * BL // 2:(h + 1) * BL // 2, c * CH:(c + 1) * CH
                ]
                eng = nc.sync if h == 0 else nc.scalar
                eng.dma_start(
                    out=dst,
                    in_=xebf[:, h * BL * T // 2:(h + 1) * BL * T // 2],
                    transpose=True,
                )

        # 4D view: x4[p, seg, b, r] = x at step seg*n + r
        x4 = x_store[:].rearrange("p (b s r) -> p s b r", s=S, r=n)

        # two segment groups: fwd slots (=segment) [0,8) and [8,15);
        # bwd slots (=segment-1) [0,7) and [7,15)
        FG = [(0, 8), (8, NP)]
        BG = [(0, 7), (7, NP)]
        # chunk order: groups 0 need chunks 0-1, groups 1 need chunks 2-3
        for c in (0, 1, 2, 3)[:nchunks]:
            emit_chunk(c)

        # ---------------- probe state + inits ----------------
        uw = cpool.tile([T, 2 * W], BF16, tag="uw")  # [u slots | w slots]
        nc.vector.tensor_scalar(
            out=uw[:, 0:BL], in0=x4[:, 0, :, 0], scalar1=expstart_col[:],
            scalar2=None, op0=Mult,
        )
        nc.vector.memset(uw[:, BL:W], 1.0)
        nc.vector.tensor_scalar(
            out=uw[:, W + (S - 2) * BL:2 * W], in0=x4[:, S - 1, :, n - 1],
            scalar1=expend_col[:], scalar2=None, op0=Mult,
        )
        # w slots 0..S-3 (segments 1..S-2) init = x at segment hi = s*n+n-1
        nc.vector.tensor_copy(
            out=uw[:, W:W + (S - 2) * BL].rearrange("p (s b) -> p s b", b=BL),
            in_=x4[:, 1:S - 1, :, n - 1],
        )

        def mm_banked(q_ap, lhsT, rhs_ap, wdt):
            # 512-col chunks: PSUM-bank-aligned (fp32 out), <= matmul max N
            for m0 in range(0, wdt, 512):
                m1 = min(m0 + 512, wdt)
                nc.tensor.matmul(
                    out=q_ap[:, m0:m1], lhsT=lhsT[:], rhs=rhs_ap[:, m0:m1],
                    start=True, stop=True,
                )

        # ---------------- probe rounds (per group) ----------------
        rho_sb = sm_pool.tile([T, W], FP32, tag="rho_sb")
        for g in range(len(FG)):
            flo, fhi = FG[g]
            blo, bhi = BG[g]
            fw = (fhi - flo) * BL
            bw = (bhi - blo) * BL
            # fwd round 0: segment 0 starts at step 1, others at step 0
            f0 = flo if flo > 0 else 1
            if f0 < fhi:
                w0 = (fhi - f0) * BL
                q0 = ps_q.tile([T, w0], FP32, space="PSUM", tag=f"q_f{g}")
                mm_banked(q0[:], E_bf, uw[:, f0 * BL:fhi * BL], w0)
                nc.vector.tensor_tensor(
                    out=uw[:, f0 * BL:fhi * BL].rearrange("p (s b) -> p s b", b=BL),
                    in0=q0[:].rearrange("p (s b) -> p s b", b=BL),
                    in1=x4[:, f0:fhi, :, 0], op=Mult,
                )
            # bwd probes for middle segments only need ~NB contraction
            # steps (error ~0.1^NB): the fwd probe carries the segment
            # scale; gamma normalizes rho's arbitrary scale exactly.
            NB = 15
            for r in range(1, n):
                qf = ps_q.tile([T, fw], FP32, space="PSUM", tag=f"q_f{g}")
                mm_banked(qf[:], E_bf, uw[:, flo * BL:fhi * BL], fw)
                nc.vector.tensor_tensor(
                    out=uw[:, flo * BL:fhi * BL].rearrange(
                        "p (s b) -> p s b", b=BL
                    ),
                    in0=qf[:].rearrange("p (s b) -> p s b", b=BL),
                    in1=x4[:, flo:fhi, :, r], op=Mult,
                )
                rb = r - 1
                if rb < NB:
                    lo2, hi2 = blo, bhi
                elif bhi == NP and rb < n - 1:
                    lo2, hi2 = NP - 1, NP  # segment S-1 runs the full length
                else:
                    continue
                bw2 = (hi2 - lo2) * BL
                qb = ps_q.tile([T, bw2], FP32, space="PSUM", tag=f"q_b{g}")
                mm_banked(qb[:], ET_bf, uw[:, W + lo2 * BL:W + hi2 * BL], bw2)
                nc.vector.tensor_tensor(
                    out=uw[:, W + lo2 * BL:W + hi2 * BL].rearrange(
                        "p (s b) -> p s b", b=BL
                    ),
                    in0=qb[:].rearrange("p (s b) -> p s b", b=BL),
                    in1=x4[:, lo2 + 1:hi2 + 1, :, n - 2 - rb], op=Mult,
                )
                if rb == NB - 1:
                    # emit rho for the short slots now (all but seg S-1 slot)
                    shi = bhi - 1 if bhi == NP else bhi
                    if shi > blo:
                        rw = (shi - blo) * BL
                        rho = ps_q.tile([T, rw], FP32, space="PSUM",
                                        tag=f"q_b{g}")
                        mm_banked(rho[:], ET_bf, uw[:, W + blo * BL:W + shi * BL], rw)
                        nc.vector.tensor_copy(
                            out=rho_sb[:, blo * BL:shi * BL], in_=rho[:]
                        )
            if bhi == NP:
                rho = ps_q.tile([T, BL], FP32, space="PSUM", tag=f"q_b{g}")
                mm_banked(rho[:], ET_bf, uw[:, W + (NP - 1) * BL:W + NP * BL], BL)
                nc.vector.tensor_copy(
                    out=rho_sb[:, (NP - 1) * BL:W], in_=rho[:]
                )

        # ---------------- combine ----------------
        # d_s = rho_s . u_{s-1} (slots aligned); gamma_s = sum_t rho_s
        prod = sm_pool.tile([T, W], FP32, tag="prod")
        nc.vector.tensor_tensor(out=prod[:], in0=rho_sb[:], in1=uw[:, 0:W], op=Mult)
        drow_ps = ps_misc.tile([1, W], FP32, space="PSUM", tag="misc")
        mm_banked(drow_ps[:], ones_col_f32, prod[:], W)
        ln_d = sm_pool.tile([1, W], FP32, tag="ln_d")
        nc.scalar.activation(out=ln_d[:], in_=drow_ps[:], func=Ln)
        grow_ps = ps_misc.tile([1, (S - 2) * BL], FP32, space="PSUM", tag="misc")
        mm_banked(grow_ps[:], ones_col_f32, rho_sb[:, 0:(S - 2) * BL], (S - 2) * BL)
        ln_g = sm_pool.tile([1, (S - 2) * BL], FP32, tag="ln_g")
        nc.scalar.activation(out=ln_g[:], in_=grow_ps[:], func=Ln)
        zred = sm_pool.tile([1, BL], FP32, tag="zred")
        nc.vector.tensor_reduce(
            out=zred[:], in_=ln_d[:].rearrange("p (s b) -> p b s", b=BL),
            axis=mybir.AxisListType.X, op=Add,
        )
        gred = sm_pool.tile([1, BL], FP32, tag="gred")
        nc.vector.tensor_reduce(
            out=gred[:], in_=ln_g[:].rearrange("p (s b) -> p b s", b=BL),
            axis=mybir.AxisListType.X, op=Add,
        )
        z_row = sm_pool.tile([1, BL], FP32, tag="z_row")
        nc.vector.tensor_tensor(out=z_row[:], in0=zred[:], in1=gred[:], op=Sub)
        nc.vector.tensor_scalar(
            out=z_row[:], in0=z_row[:], scalar1=float((L - 1) * 7 * np.log(2.0)),
            scalar2=None, op0=Add,
        )
        nc.sync.dma_start(out=out_z[:], in_=z_row[:])

    # Postamble: drain + clear semaphores so the NEFF is re-executable
    # (without target_bir_lowering there is no preamble sem_clear).
    nc.reset()
    return nc


def _split_multi_waits(nc):
    """Workaround: this walrus encodes at most ONE sync-wait per instruction
    ("Too many sync wait commands"). Move extra waits onto same-engine NoOps
    inserted immediately before the instruction (engine blocks on each in
    program order, so semantics are identical)."""
    for fn in nc.m.functions:
        for bb in fn.blocks:
            insts = bb.instructions
            i = 0
            while i < len(insts):
                inst = insts[i]
                si = inst.sync_info
                if si is not None and si.on_wait and len(si.on_wait) > 1:
                    waits = list(si.on_wait)
                    for k, wsync in enumerate(waits[:-1]):
                        nop = mybir.InstNoOp(
                            name=f"{inst.name}-w{k}",
                            engine=inst.engine,
                            ins=[],
                            outs=[],
                            sync_info=mybir.SyncInfo(on_wait=[wsync], on_update=[]),
                        )
                        insts.insert(i, nop)
                        i += 1
                    inst.sync_info = mybir.SyncInfo(
                        on_wait=[waits[-1]], on_update=list(si.on_update or [])
                    )
                i += 1
    return nc


_NC_CACHE = {}


def _get_nc():
    key = "full"
    if key not in _NC_CACHE:
        builder = (
            build_crf_kernel_v2
            if int(os.environ.get("CRF_V2", "1"))
            else build_crf_kernel
        )
        _NC_CACHE[key] = _split_multi_waits(builder())
    return _NC_CACHE[key]


def make_in_maps(emissions, tags, start_transitions, end_transitions, transitions):
    emissions = np.ascontiguousarray(np.asarray(emissions, dtype=np.float32))
    tags = np.ascontiguousarray(np.asarray(tags).astype(np.int32))
    start = np.ascontiguousarray(
        np.asarray(start_transitions, dtype=np.float32).reshape(T, 1)
    )
    end = np.ascontiguousarray(
        np.asarray(end_transitions, dtype=np.float32).reshape(T, 1)
    )
    trans = np.ascontiguousarray(np.asarray(transitions, dtype=np.float32))
    in_maps = []
    for i in range(NCORES):
        sl = slice(i * BL, (i + 1) * BL)
        in_maps.append({
            "emissions": np.ascontiguousarray(emissions[:, sl, :]),
            "tags": np.ascontiguousarray(tags[:, sl]),
            "start_t": start,
            "end_t": end,
            "trans": trans,
        })
    return in_maps


def combine_outputs(results):
    log_den = 0.0
    log_num = 0.0
    for res in results:
        log_den += np.asarray(res["out_z"], dtype=np.float64).sum()
        log_num += np.asarray(res["out_gold"], dtype=np.float64).sum()
        log_num += np.asarray(res["out_trans"], dtype=np.float64).sum()
        log_num += np.asarray(res["out_se"], dtype=np.float64).sum()
    return np.float32((log_den - log_num) / B)


def kernel(emissions, tags, mask, start_transitions, end_transitions, transitions):
    mask = np.asarray(mask)
    assert mask.all(), "kernel assumes mask of all ones (spec fill=ones)"
    from concourse.bass_utils import run_bass_kernel_spmd

    nc = _get_nc()
    in_maps = make_in_maps(
        emissions, tags, start_transitions, end_transitions, transitions
    )
    trace = bool(int(os.environ.get("CRF_TRACE", "0")))
    if trace:
        try:
            import importlib.util as _iu

            try:
                from antenv import axon_hooks as _hooks
            except ImportError:
                # antenv may already be cached from a copy lacking axon_hooks;
                # load ours by path and graft it into the package.
                import antenv

                _spec = _iu.spec_from_file_location(
                    "antenv.axon_hooks", "/opt/trn_rl_repo/antenv/axon_hooks.py"
                )
                _hooks = _iu.module_from_spec(_spec)
                _spec.loader.exec_module(_hooks)
                sys.modules["antenv.axon_hooks"] = _hooks
                antenv.axon_hooks = _hooks

            if _hooks.get_axon_ntff_profile_hook() is None:
                from trn_agent_boot.trn_boot import _ntff_profile_via_ctypes

                _hooks.set_axon_ntff_profile_hook(
                    _ntff_profile_via_ctypes("/opt/axon/libaxon_pjrt.so")
                )
        except Exception as e:  # profiling is best-effort
            print(f"NTFF hook install failed ({e}); running untraced")
            trace = False
    if kernel._calls > 0:
        # Re-executing a cached NEFF is unreliable (semaphore state);
        # force a fresh PJRT executable (NEFF compile is disk-cached).
        import jax

        jax.clear_caches()
    kernel._calls += 1
    br = run_bass_kernel_spmd(nc, in_maps, list(range(NCORES)), trace=trace)
    if trace and br.exec_time_ns is not None:
        print(f"HW exec time: {br.exec_time_ns} ns")
        kernel.last_exec_time_ns = br.exec_time_ns
    return combine_outputs(br.results)


kernel.last_exec_time_ns = None
kernel._calls = 0


# revision 26
# speedup vs baseline: 2.7063x; 1.0026x over previous
"""CRF loss (negative log-likelihood) kernel for Trainium2, 8 NeuronCores.

Strategy (data-parallel over batch, per the sharding hint):
  - Each of 8 cores gets B/8 = 64 sequences; the same NEFF runs SPMD on all
    cores with per-core input shards, and the host sums the tiny partials.
  - Denominator (log partition, the heavy part): the forward recursion
    p_i = diag(x_i) E^T p_{i-1} (x = exp(emissions), E = exp(transitions))
    is a product of positive matrices, which contracts projectively
    (Birkhoff) by ~tanh(0.1) per step since |transitions| <= 0.1. A 32-step
    segment map is therefore numerically rank-1, so the 511-step serial
    chain splits into 16 independent segments evaluated with forward
    probes u_s = M_s w (full length, carries the scale) and backward
    probes rho_s ~ M_s^T z (16 steps suffice), recombined exactly via
      Z_b = (rho_{S-1}.u_{S-2}) * prod_s (rho_s.u_{s-1}) / (rho_s.w).
    Segments run as wide (128 x 960) matmul+multiply rounds — latency
    chains are 32 long instead of 511. A 2^-7 scale folded into E keeps
    the exp-domain values in range (compensated by +511*7*ln2).
  - Emissions stream: SWDGE DMA casts fp32->bf16 in a (step, b*t) layout
    (32KB contiguous per partition), batched ACT exp, then one xbar
    transpose-DMA per 128-step chunk (3D out AP) into x[t, b*L+i].
    Probes run in two segment groups so the scan overlaps the stream.
  - Numerator (gold path score) via indirect DMA element gathers:
    emissions at gold tags, transitions at tag pairs, start/end; reduced
    on device. bf16 is safe for the denominator because the loss gradient
    w.r.t. emissions is bounded (errors average out); the numerator reads
    raw fp32 values.
"""

import os
import sys

import numpy as np

for _p in ("/opt/trn_rl_repo", os.path.expanduser("~/.axon_site/_ro/trn_rl_repo")):
    if os.path.isdir(_p):
        if _p not in sys.path:
            sys.path.insert(0, _p)
        break

import concourse.bass as bass  # noqa: E402
from concourse import mybir  # noqa: E402
from concourse.masks import make_identity  # noqa: E402
from concourse.tile import TileContext  # noqa: E402
from concourse.tile_rust import add_dep_helper  # noqa: E402

FP32 = mybir.dt.float32
BF16 = mybir.dt.bfloat16
I32 = mybir.dt.int32
Exp = mybir.ActivationFunctionType.Exp
Ln = mybir.ActivationFunctionType.Ln
Add = mybir.AluOpType.add
Sub = mybir.AluOpType.subtract
Mult = mybir.AluOpType.mult

L, B, T = 512, 512, 128
NCORES = 8
BL = B // NCORES  # 64 sequences per core


def build_crf_kernel(L=L, BL=BL, T=T, CH=32, RENORM=64):
    """Build the per-core Bass kernel (SPMD: same NEFF, different inputs)."""
    assert L % CH == 0 and CH % 2 == 0
    nchunks = L // CH
    MID = L // 2  # fwd covers steps 1..MID, bwd covers MID+1..L-1
    TCH = min(128, L)  # tags chunk (steps on partitions)
    ntch = (L + TCH - 1) // TCH
    GW = BL  # free width contributed per tags chunk in the gather tiles

    nc = bass.Bass()

    emis = nc.declare_dram_parameter("emissions", [L, BL, T], FP32, isOutput=False)
    tags = nc.declare_dram_parameter("tags", [L, BL], I32, isOutput=False)
    start_t = nc.declare_dram_parameter("start_t", [T, 1], FP32, isOutput=False)
    end_t = nc.declare_dram_parameter("end_t", [T, 1], FP32, isOutput=False)
    trans = nc.declare_dram_parameter("trans", [T, T], FP32, isOutput=False)
    out_z = nc.declare_dram_parameter("out_z", [1, BL], FP32, isOutput=True)
    out_gold = nc.declare_dram_parameter("out_gold", [TCH, 1], FP32, isOutput=True)
    out_trans = nc.declare_dram_parameter("out_trans", [TCH, 1], FP32, isOutput=True)
    out_se = nc.declare_dram_parameter("out_se", [16, 8], FP32, isOutput=True)

    from contextlib import ExitStack

    with TileContext(nc) as tc, ExitStack() as es:
        cpool = es.enter_context(tc.tile_pool(name="consts", bufs=1))
        ebf_pool = es.enter_context(tc.tile_pool(name="ebf", bufs=2))
        xtr_pool = es.enter_context(tc.tile_pool(name="xtraw", bufs=2))
        xf_pool = es.enter_context(tc.tile_pool(name="x_f", bufs=3))
        xb_pool = es.enter_context(tc.tile_pool(name="x_b", bufs=3))
        p_pool = es.enter_context(tc.tile_pool(name="pp", bufs=4))
        sm_pool = es.enter_context(tc.tile_pool(name="small", bufs=2))
        num_pool = es.enter_context(tc.tile_pool(name="numer", bufs=1))
        tg_pool = es.enter_context(tc.tile_pool(name="tagt", bufs=2))
        ps_q = es.enter_context(tc.tile_pool(name="ps_q", bufs=2, space="PSUM"))
        ps_misc = es.enter_context(tc.tile_pool(name="ps_misc", bufs=1, space="PSUM"))

        # ---------------- constants ----------------
        trans_sb = cpool.tile([T, T], FP32, tag="trans_sb")
        nc.sync.dma_start(out=trans_sb[:], in_=trans[:])
        ident = cpool.tile([128, 128], FP32, tag="ident")
        make_identity(nc, ident[:])
        # Fold a 2^-7 scale into E so per-step mass growth is ~1 (the
        # sum over 128 source tags would otherwise overflow in ~16 steps).
        # Compensated exactly by +(L-1)*7*ln2 on the final log-partition.
        LOG_SCALE = -7.0 * float(np.log(2.0))
        lsc_col = cpool.tile([128, 1], FP32, tag="lsc_col")
        nc.vector.memset(lsc_col[:], LOG_SCALE)
        E_bf = cpool.tile([T, T], BF16, tag="E_bf")
        nc.scalar.activation(out=E_bf[:], in_=trans_sb[:], func=Exp, bias=lsc_col[:])
        transT_ps = ps_misc.tile([T, T], FP32, space="PSUM", tag="transT")
        nc.tensor.transpose(out=transT_ps[:], in_=trans_sb[:], identity=ident[:])
        ET_bf = cpool.tile([T, T], BF16, tag="ET_bf")
        nc.scalar.activation(
            out=ET_bf[:], in_=transT_ps[:], func=Exp, bias=lsc_col[:]
        )

        start_col = cpool.tile([T, 1], FP32, tag="start_col")
        nc.sync.dma_start(out=start_col[:], in_=start_t[:])
        end_col = cpool.tile([T, 1], FP32, tag="end_col")
        nc.sync.dma_start(out=end_col[:], in_=end_t[:])

        ones_col_bf = cpool.tile([128, 1], BF16, tag="ones_col_bf")
        nc.vector.memset(ones_col_bf[:], 1.0)
        ones_col_f32 = cpool.tile([128, 1], FP32, tag="ones_col_f32")
        nc.vector.memset(ones_col_f32[:], 1.0)
        ones_row_bf = cpool.tile([1, 128], BF16, tag="ones_row_bf")
        nc.vector.memset(ones_row_bf[:], 1.0)
        ones_bl_bf = cpool.tile([128, BL], BF16, tag="ones_bl_bf")
        nc.vector.memset(ones_bl_bf[:], 1.0)

        c_f = sm_pool.tile([1, BL], FP32, tag="c_f")
        nc.vector.memset(c_f[:], 0.0)
        c_b = sm_pool.tile([1, BL], FP32, tag="c_b")
        nc.vector.memset(c_b[:], 0.0)

        # ---------------- numerator: gathers ----------------
        gold_idx = num_pool.tile([TCH, L * BL // TCH], I32, tag="gold_idx")
        trans_idx = num_pool.tile([TCH, L * BL // TCH], I32, tag="trans_idx")
        tags_cur = {}
        for c in range(ntch):
            tcur = tg_pool.tile([TCH, BL], I32, tag="tags_cur")
            nc.sync.dma_start(out=tcur[:], in_=tags[c * TCH:(c + 1) * TCH, :])
            tags_cur[c] = tcur
            gsl = gold_idx[:, c * GW:(c + 1) * GW]
            # gold flat index = (i*BL + b)*T + tags[i, b]
            nc.gpsimd.iota(
                gsl, pattern=[[T, BL]], base=c * TCH * BL * T,
                channel_multiplier=BL * T,
            )
            nc.vector.tensor_tensor(out=gsl, in0=gsl, in1=tcur[:], op=Add)

            tprev = tg_pool.tile([TCH, BL], I32, tag="tags_prev")
            if c == 0:
                nc.vector.memset(tprev[0:1, :], 0)
                nc.sync.dma_start(out=tprev[1:TCH, :], in_=tags[0:TCH - 1, :])
            else:
                nc.sync.dma_start(
                    out=tprev[:], in_=tags[c * TCH - 1:(c + 1) * TCH - 1, :]
                )
            tsl = trans_idx[:, c * GW:(c + 1) * GW]
            # trans flat index = tags[i-1]*T + tags[i]
            nc.vector.tensor_scalar(
                out=tsl, in0=tprev[:], scalar1=T, scalar2=None, op0=Mult
            )
            nc.vector.tensor_tensor(out=tsl, in0=tsl, in1=tcur[:], op=Add)
        # pair step 0 does not exist: poison its indices; bounds_check skips them
        nc.vector.memset(trans_idx[0:1, 0:GW], 1 << 24)

        gvals = num_pool.tile([TCH, L * BL // TCH], FP32, tag="gvals")
        nc.vector.memset(gvals[:], 0.0)  # OOB-skipped entries leave SBUF as-is
        nc.gpsimd.indirect_dma_start(
            out=gvals[:], out_offset=None, in_=emis[:],
            in_offset=bass.IndirectOffsetOnAxis(ap=gold_idx[:], axis=2),
            bounds_check=L * BL * T - 1, oob_is_err=False,
        )
        tvals = num_pool.tile([TCH, L * BL // TCH], FP32, tag="tvals")
        nc.vector.memset(tvals[:], 0.0)  # OOB-skipped entries leave SBUF as-is
        nc.gpsimd.indirect_dma_start(
            out=tvals[:], out_offset=None, in_=trans[:],
            in_offset=bass.IndirectOffsetOnAxis(ap=trans_idx[:], axis=1),
            bounds_check=T * T - 1, oob_is_err=False,
        )
        gold_red = num_pool.tile([TCH, 1], FP32, tag="gold_red")
        nc.vector.tensor_reduce(
            out=gold_red[:], in_=gvals[:], axis=mybir.AxisListType.X, op=Add
        )
        trans_red = num_pool.tile([TCH, 1], FP32, tag="trans_red")
        nc.vector.tensor_reduce(
            out=trans_red[:], in_=tvals[:], axis=mybir.AxisListType.X, op=Add
        )
        nc.sync.dma_start(out=out_gold[:], in_=gold_red[:])
        nc.sync.dma_start(out=out_trans[:], in_=trans_red[:])

        # start/end transition gathers (64 each)
        se_idx = num_pool.tile([16, 8], I32, tag="se_idx")
        nc.sync.dma_start(
            out=se_idx[:, 0:4], in_=tags[0:1, :].rearrange("o (p j) -> (o p) j", p=16)
        )
        nc.sync.dma_start(
            out=se_idx[:, 4:8],
            in_=tags[L - 1:L, :].rearrange("o (p j) -> (o p) j", p=16),
        )
        se_vals = num_pool.tile([16, 8], FP32, tag="se_vals")
        nc.gpsimd.indirect_dma_start(
            out=se_vals[:, 0:4], out_offset=None, in_=start_t[:],
            in_offset=bass.IndirectOffsetOnAxis(ap=se_idx[:, 0:4], axis=1),
            bounds_check=T - 1, oob_is_err=False,
        )
        nc.gpsimd.indirect_dma_start(
            out=se_vals[:, 4:8], out_offset=None, in_=end_t[:],
            in_offset=bass.IndirectOffsetOnAxis(ap=se_idx[:, 4:8], axis=1),
            bounds_check=T - 1, oob_is_err=False,
        )
        nc.sync.dma_start(out=out_se[:], in_=se_vals[:])

        # ---------------- emissions stream: cast + transpose + exp ----------------
        H = CH // 2
        x_tiles = {}      # chunk -> x tile (exp'ed, (t, b) layout)
        xtraw_tiles = {}  # chunk -> pre-exp transposed tile (for biased inits)

        def emit_chunk(c, pool):
            ebf = ebf_pool.tile([2 * BL, H * T], BF16, tag="ebf")
            for h in range(2):
                src = emis[c * CH + h * H:c * CH + (h + 1) * H, :, :].rearrange(
                    "j b t -> b j t"
                )
                dst = ebf[h * BL:(h + 1) * BL, :].rearrange("b (j t) -> b j t", j=H)
                nc.gpsimd.dma_start(out=dst, in_=src)  # fp32 -> bf16 cast in DMA
            xtraw = xtr_pool.tile([T, CH * BL], BF16, tag="xtraw")
            for j in range(H):
                nc.sync.dma_start(
                    out=xtraw[:, j * 2 * BL:(j + 1) * 2 * BL],
                    in_=ebf[:, j * T:(j + 1) * T],
                    transpose=True,
                )
            x = pool.tile([T, CH * BL], BF16, tag=pool.name)
            nbat = (CH * BL + 511) // 512
            for k in range(nbat):
                sl = slice(k * 512, min((k + 1) * 512, CH * BL))
                nc.scalar.activation(out=x[:, sl], in_=xtraw[:, sl], func=Exp)
            x_tiles[c] = x
            xtraw_tiles[c] = xtraw

        def x_slice(i, raw=False):
            c, o = i // CH, i % CH
            t = (xtraw_tiles if raw else x_tiles)[c]
            col = o * 2 * BL if o < H else (o - H) * 2 * BL + BL
            return t[:, col:col + BL]

        nfwd_chunks = MID // CH + 1  # fwd consumes chunks 0 .. MID//CH (x_MID)
        for s in range(max(nfwd_chunks, nchunks - nfwd_chunks + 1)):
            cf, cb = s, nchunks - 1 - s
            if cf < nfwd_chunks:
                emit_chunk(cf, xf_pool)
            if cb >= nfwd_chunks and cb != cf:
                emit_chunk(cb, xb_pool)

        # ---------------- scan init ----------------
        # p_0 = exp(e_0 + start), w_{L-1} = exp(e_{L-1} + end)
        p_prev = p_pool.tile([T, BL], BF16, tag="p_f")
        nc.scalar.activation(
            out=p_prev[:], in_=x_slice(0, raw=True), func=Exp, bias=start_col[:]
        )
        w_prev = p_pool.tile([T, BL], BF16, tag="p_b")
        nc.scalar.activation(
            out=w_prev[:], in_=x_slice(L - 1, raw=True), func=Exp, bias=end_col[:]
        )

        def renorm(p_cur, c_row, tag):
            s_ps = ps_misc.tile([1, BL], FP32, space="PSUM", tag="s_ps")
            nc.tensor.matmul(
                out=s_ps[:], lhsT=ones_col_bf[:], rhs=p_cur[:], start=True, stop=True
            )
            rec32 = sm_pool.tile([1, BL], FP32, tag="rec32")
            nc.vector.reciprocal(out=rec32[:], in_=s_ps[:])
            recbf = sm_pool.tile([1, BL], BF16, tag="recbf")
            nc.vector.tensor_copy(out=recbf[:], in_=rec32[:])
            lnr = sm_pool.tile([1, BL], FP32, tag="lnr")
            nc.scalar.activation(out=lnr[:], in_=recbf[:], func=Ln)
            nc.vector.tensor_tensor(out=c_row[:], in0=c_row[:], in1=lnr[:], op=Sub)
            bc_ps = ps_misc.tile([128, BL], FP32, space="PSUM", tag="bc_ps")
            nc.tensor.matmul(
                out=bc_ps[:], lhsT=ones_row_bf[:], rhs=recbf[:], start=True, stop=True
            )
            p_new = p_pool.tile([T, BL], BF16, tag=tag)
            nc.vector.tensor_tensor(out=p_new[:], in0=bc_ps[:], in1=p_cur[:], op=Mult)
            return p_new

        # ---------------- interleaved forward/backward rounds ----------------
        # fwd round i (1..MID):    p_i = (E^T p_{i-1}) * x_i
        # bwd round j (L-1..MID+2): w_{j-1} = x_{j-1} * (E w_j)
        # final bwd matmul (j=MID+1) leaves v_MID = E w_{MID+1} in PSUM.
        nfwd = MID
        nbwd = L - 1 - MID  # matmul count; last one has no multiply
        v_mid_ps = None
        for r in range(max(nfwd, nbwd)):
            if r < nfwd:
                i = r + 1
                qf = ps_q.tile([T, BL], FP32, space="PSUM", tag="qf")
                nc.tensor.matmul(
                    out=qf[:], lhsT=E_bf[:], rhs=p_prev[:], start=True, stop=True
                )
                p_new = p_pool.tile([T, BL], BF16, tag="p_f")
                nc.vector.tensor_tensor(
                    out=p_new[:], in0=qf[:], in1=x_slice(i), op=Mult
                )
                p_prev = p_new
                if i % RENORM == 0 and i < nfwd:
                    p_prev = renorm(p_prev, c_f, "p_f")
            if r < nbwd:
                j = L - 1 - r
                qb = ps_q.tile([T, BL], FP32, space="PSUM", tag="qb")
                nc.tensor.matmul(
                    out=qb[:], lhsT=ET_bf[:], rhs=w_prev[:], start=True, stop=True
                )
                if j == MID + 1:
                    v_mid_ps = qb
                else:
                    w_new = p_pool.tile([T, BL], BF16, tag="p_b")
                    nc.vector.tensor_tensor(
                        out=w_new[:], in0=qb[:], in1=x_slice(j - 1), op=Mult
                    )
                    w_prev = w_new
                    if r % RENORM == RENORM // 2 and r < nbwd - 2:
                        w_prev = renorm(w_prev, c_b, "p_b")

        # ---------------- combine: logZ = ln(sum_t p_MID * v_MID) + c_f + c_b ----
        prod = sm_pool.tile([T, BL], FP32, tag="prod")
        nc.vector.tensor_tensor(
            out=prod[:], in0=v_mid_ps[:], in1=p_prev[:], op=Mult
        )
        zsum_ps = ps_misc.tile([1, BL], FP32, space="PSUM", tag="zsum")
        nc.tensor.matmul(
            out=zsum_ps[:], lhsT=ones_col_f32[:], rhs=prod[:], start=True, stop=True
        )
        z_row = sm_pool.tile([1, BL], FP32, tag="z_row")
        nc.scalar.activation(out=z_row[:], in_=zsum_ps[:], func=Ln)
        nc.vector.tensor_tensor(out=z_row[:], in0=z_row[:], in1=c_f[:], op=Add)
        nc.vector.tensor_tensor(out=z_row[:], in0=z_row[:], in1=c_b[:], op=Add)
        # compensate the 2^-7 folded into E: (L-1) matmuls total
        nc.vector.tensor_scalar(
            out=z_row[:], in0=z_row[:], scalar1=float((L - 1) * 7 * np.log(2.0)),
            scalar2=None, op0=Add,
        )
        nc.sync.dma_start(out=out_z[:], in_=z_row[:])

    # Postamble: drain + clear semaphores so the NEFF is re-executable
    # (without target_bir_lowering there is no preamble sem_clear).
    nc.reset()
    return nc




def build_crf_kernel_v2(L=L, BL=BL, T=T, S=16):
    """v2/v3: segmented scan via rank-1 probe decomposition.

    Products of positive matrices contract projectively (Birkhoff): each
    step map D_x E^T shrinks Hilbert-metric diameter by ~tanh(0.1) (since
    |transitions| <= 0.1), so a 32-step segment map is rank-1 to ~1e-32.
    Each segment is evaluated independently with a forward probe
    u_s = M_s w and a backward probe rho_s = M_s^T z; the log-partition
    telescopes into per-segment scalars:

      Z_b = (rho_{S-1} . u_{S-2}) * prod_{s=1}^{S-2} (rho_s . u_{s-1}) / g_s
      g_s = sum_t u_s[t]

    with u_0 seeded exactly with p_0 = exp(start + e_0) and rho_{S-1}
    seeded with exp(end). This removes the 511-step serial latency chain:
    only n = L/S rounds of wide ops remain. Probes run in two segment
    groups so the second half of the emissions stream overlaps the first
    group's scan.

    Emissions stream: SWDGE cast-DMA in (step, b*t) layout (32KB
    contiguous per partition), batched ACT exp, then ONE xbar
    transpose-DMA per 128-step chunk using a 3D out AP (out[t,b,i] =
    in[i, b*T+t]) into x_store[t, b*L + i].
    """
    assert L % S == 0
    n = L // S
    CH = 128                     # steps per emissions chunk (partition dim)
    nchunks = L // CH
    segs_per_chunk = CH // n
    NP = S - 1
    W = NP * BL
    TCH = min(128, L)
    ntch = (L + TCH - 1) // TCH
    GW = BL

    nc = bass.Bass()

    emis = nc.declare_dram_parameter("emissions", [L, BL, T], FP32, isOutput=False)
    tags = nc.declare_dram_parameter("tags", [L, BL], I32, isOutput=False)
    start_t = nc.declare_dram_parameter("start_t", [T, 1], FP32, isOutput=False)
    end_t = nc.declare_dram_parameter("end_t", [T, 1], FP32, isOutput=False)
    trans = nc.declare_dram_parameter("trans", [T, T], FP32, isOutput=False)
    out_z = nc.declare_dram_parameter("out_z", [1, BL], FP32, isOutput=True)
    out_gold = nc.declare_dram_parameter("out_gold", [TCH, 1], FP32, isOutput=True)
    out_trans = nc.declare_dram_parameter("out_trans", [TCH, 1], FP32, isOutput=True)
    out_se = nc.declare_dram_parameter("out_se", [16, 8], FP32, isOutput=True)

    from contextlib import ExitStack

    with TileContext(nc) as tc, ExitStack() as es:
        cpool = es.enter_context(tc.tile_pool(name="consts", bufs=1))
        ebf_pool = es.enter_context(tc.tile_pool(name="ebf", bufs=2))
        xe_pool = es.enter_context(tc.tile_pool(name="xebf", bufs=2))
        sm_pool = es.enter_context(tc.tile_pool(name="small", bufs=2))
        num_pool = es.enter_context(tc.tile_pool(name="numer", bufs=1))
        tg_pool = es.enter_context(tc.tile_pool(name="tagt", bufs=2))
        ps_q = es.enter_context(tc.tile_pool(name="ps_q", bufs=1, space="PSUM"))
        ps_misc = es.enter_context(tc.tile_pool(name="ps_misc", bufs=1, space="PSUM"))

        # ---------------- constants ----------------
        trans_sb = cpool.tile([T, T], FP32, tag="trans_sb")
        nc.sync.dma_start(out=trans_sb[:], in_=trans[:])
        ident = cpool.tile([128, 128], FP32, tag="ident")
        make_identity(nc, ident[:])
        # Fold 2^-7 into E so per-step mass growth is ~1 (compensated by
        # +(L-1)*7*ln2 at the end); otherwise the 128-way sum overflows.
        LOG_SCALE = -7.0 * float(np.log(2.0))
        lsc_col = cpool.tile([128, 1], FP32, tag="lsc_col")
        nc.vector.memset(lsc_col[:], LOG_SCALE)
        E_bf = cpool.tile([T, T], BF16, tag="E_bf")
        nc.scalar.activation(out=E_bf[:], in_=trans_sb[:], func=Exp, bias=lsc_col[:])
        transT_ps = ps_misc.tile([T, T], FP32, space="PSUM", tag="misc")
        nc.tensor.transpose(out=transT_ps[:], in_=trans_sb[:], identity=ident[:])
        ET_bf = cpool.tile([T, T], BF16, tag="ET_bf")
        nc.scalar.activation(
            out=ET_bf[:], in_=transT_ps[:], func=Exp, bias=lsc_col[:]
        )
        start_col = cpool.tile([T, 1], FP32, tag="start_col")
        nc.sync.dma_start(out=start_col[:], in_=start_t[:])
        end_col = cpool.tile([T, 1], FP32, tag="end_col")
        nc.sync.dma_start(out=end_col[:], in_=end_t[:])
        expstart_col = cpool.tile([T, 1], FP32, tag="expstart_col")
        nc.scalar.activation(out=expstart_col[:], in_=start_col[:], func=Exp)
        expend_col = cpool.tile([T, 1], FP32, tag="expend_col")
        nc.scalar.activation(out=expend_col[:], in_=end_col[:], func=Exp)
        ones_col_f32 = cpool.tile([128, 1], FP32, tag="ones_col_f32")
        nc.vector.memset(ones_col_f32[:], 1.0)
        ones_col_bf = cpool.tile([128, 1], BF16, tag="ones_col_bf")
        nc.vector.memset(ones_col_bf[:], 1.0)

        # ---------------- numerator (indirect gathers) ----------------
        gold_idx = num_pool.tile([TCH, L * BL // TCH], I32, tag="gold_idx")
        trans_idx = num_pool.tile([TCH, L * BL // TCH], I32, tag="trans_idx")
        for c in range(ntch):
            tcur = tg_pool.tile([TCH, BL], I32, tag="tags_cur")
            nc.sync.dma_start(out=tcur[:], in_=tags[c * TCH:(c + 1) * TCH, :])
            gsl = gold_idx[:, c * GW:(c + 1) * GW]
            nc.gpsimd.iota(
                gsl, pattern=[[T, BL]], base=c * TCH * BL * T,
                channel_multiplier=BL * T,
            )
            nc.vector.tensor_tensor(out=gsl, in0=gsl, in1=tcur[:], op=Add)
            tprev = tg_pool.tile([TCH, BL], I32, tag="tags_prev")
            if c == 0:
                nc.vector.memset(tprev[0:1, :], 0)
                nc.sync.dma_start(out=tprev[1:TCH, :], in_=tags[0:TCH - 1, :])
            else:
                nc.sync.dma_start(
                    out=tprev[:], in_=tags[c * TCH - 1:(c + 1) * TCH - 1, :]
                )
            tsl = trans_idx[:, c * GW:(c + 1) * GW]
            nc.vector.tensor_scalar(
                out=tsl, in0=tprev[:], scalar1=T, scalar2=None, op0=Mult
            )
            nc.vector.tensor_tensor(out=tsl, in0=tsl, in1=tcur[:], op=Add)
        nc.vector.memset(trans_idx[0:1, 0:GW], 1 << 24)

        gvals = num_pool.tile([TCH, L * BL // TCH], FP32, tag="gvals")
        nc.vector.memset(gvals[:], 0.0)  # OOB-skipped entries leave SBUF as-is
        nc.gpsimd.indirect_dma_start(
            out=gvals[:], out_offset=None, in_=emis[:],
            in_offset=bass.IndirectOffsetOnAxis(ap=gold_idx[:], axis=2),
            bounds_check=L * BL * T - 1, oob_is_err=False,
        )
        tvals = num_pool.tile([TCH, L * BL // TCH], FP32, tag="tvals")
        nc.vector.memset(tvals[:], 0.0)  # OOB-skipped entries leave SBUF as-is
        nc.gpsimd.indirect_dma_start(
            out=tvals[:], out_offset=None, in_=trans[:],
            in_offset=bass.IndirectOffsetOnAxis(ap=trans_idx[:], axis=1),
            bounds_check=T * T - 1, oob_is_err=False,
        )
        gold_red = num_pool.tile([TCH, 1], FP32, tag="gold_red")
        nc.vector.tensor_reduce(
            out=gold_red[:], in_=gvals[:], axis=mybir.AxisListType.X, op=Add
        )
        trans_red = num_pool.tile([TCH, 1], FP32, tag="trans_red")
        nc.vector.tensor_reduce(
            out=trans_red[:], in_=tvals[:], axis=mybir.AxisListType.X, op=Add
        )
        nc.sync.dma_start(out=out_gold[:], in_=gold_red[:])
        nc.sync.dma_start(out=out_trans[:], in_=trans_red[:])

        se_idx = num_pool.tile([16, 8], I32, tag="se_idx")
        nc.sync.dma_start(
            out=se_idx[:, 0:4], in_=tags[0:1, :].rearrange("o (p j) -> (o p) j", p=16)
        )
        nc.sync.dma_start(
            out=se_idx[:, 4:8],
            in_=tags[L - 1:L, :].rearrange("o (p j) -> (o p) j", p=16),
        )
        se_vals = num_pool.tile([16, 8], FP32, tag="se_vals")
        nc.gpsimd.indirect_dma_start(
            out=se_vals[:, 0:4], out_offset=None, in_=start_t[:],
            in_offset=bass.IndirectOffsetOnAxis(ap=se_idx[:, 0:4], axis=1),
            bounds_check=T - 1, oob_is_err=False,
        )
        nc.gpsimd.indirect_dma_start(
            out=se_vals[:, 4:8], out_offset=None, in_=end_t[:],
            in_offset=bass.IndirectOffsetOnAxis(ap=se_idx[:, 4:8], axis=1),
            bounds_check=T - 1, oob_is_err=False,
        )
        nc.sync.dma_start(out=out_se[:], in_=se_vals[:])

        # -------- emissions: cast + exp + one batched xbar transpose/chunk ------
        x_store = cpool.tile([T, BL * L], BF16, tag="x_store")  # [t, b*L + i]
        xp_insts = {}

        def emit_chunk(c):
            ebf = ebf_pool.tile([CH, BL * T], BF16, tag="ebf")
            nc.gpsimd.dma_start(
                out=ebf[:],
                in_=emis[c * CH:(c + 1) * CH, :, :].rearrange("i b t -> i (b t)"),
            )  # fp32->bf16 cast; 32KB contiguous per partition
            xebf = xe_pool.tile([CH, BL * T], BF16, tag="xebf")
            for k in range(4):
                sl = slice(k * BL * T // 4, (k + 1) * BL * T // 4)
                nc.scalar.activation(out=xebf[:, sl], in_=ebf[:, sl], func=Exp)
            # out[t, b, i] = in[i, b*T + t] for this chunk's i-range;
            # split by b-half across both HWDGE rings (SP + ACT) to halve
            # the serial xbar time.
            for h in range(2):
                dst = x_store[:].rearrange("p (b l) -> p b l", l=L)[
                    :, h * BL // 2:(h + 1) * BL // 2, c * CH:(c + 1) * CH
                ]
                eng = nc.sync if h == 0 else nc.scalar
                xp_insts[(c, h)] = eng.dma_start(
                    out=dst,
                    in_=xebf[:, h * BL * T // 2:(h + 1) * BL * T // 2],
                    transpose=True,
                )

        # 4D view: x4[p, seg, b, r] = x at step seg*n + r
        x4 = x_store[:].rearrange("p (b s r) -> p s b r", s=S, r=n)

        # two segment groups: fwd slots (=segment) [0,8) and [8,15);
        # bwd slots (=segment-1) [0,7) and [7,15)
        FG = [(0, 8), (8, NP)]
        BG = [(0, 7), (7, NP)]
        # chunk order: groups 0 need chunks 0-1, groups 1 need chunks 2-3
        for c in (0, 1, 2, 3)[:nchunks]:
            emit_chunk(c)

        # ---------------- probe state + inits ----------------
        uw = cpool.tile([T, 2 * W], BF16, tag="uw")  # [u slots | w slots]
        i0 = nc.vector.tensor_scalar(
            out=uw[:, 0:BL], in0=x4[:, 0, :, 0], scalar1=expstart_col[:],
            scalar2=None, op0=Mult,
        )
        nc.vector.memset(uw[:, BL:W], 1.0)
        i1 = nc.vector.tensor_scalar(
            out=uw[:, W + (S - 2) * BL:2 * W], in0=x4[:, S - 1, :, n - 1],
            scalar1=expend_col[:], scalar2=None, op0=Mult,
        )
        # w slots 0..S-3 (segments 1..S-2) init = x at segment hi = s*n+n-1
        i2 = nc.vector.tensor_copy(
            out=uw[:, W:W + (S - 2) * BL].rearrange("p (s b) -> p s b", b=BL),
            in_=x4[:, 1:S - 1, :, n - 1],
        )
        # Belt-and-braces: explicit edges from x readers to the transposes
        # (the strided-AP overlap math is the one thing we don't fully trust).
        for h in range(2):
            add_dep_helper(i0.ins, xp_insts[(0, h)].ins, reason="x ready")
            add_dep_helper(i1.ins, xp_insts[(nchunks - 1, h)].ins, reason="x ready")
            for c in range(nchunks):
                add_dep_helper(i2.ins, xp_insts[(c, h)].ins, reason="x ready")

        def mm_banked(q_ap, lhsT, rhs_ap, wdt):
            # 512-col chunks: PSUM-bank-aligned (fp32 out), <= matmul max N
            for m0 in range(0, wdt, 512):
                m1 = min(m0 + 512, wdt)
                nc.tensor.matmul(
                    out=q_ap[:, m0:m1], lhsT=lhsT[:], rhs=rhs_ap[:, m0:m1],
                    start=True, stop=True,
                )

        # ---------------- probe rounds (per group) ----------------
        rho_sb = sm_pool.tile([T, W], FP32, tag="rho_sb")
        for g in range(len(FG)):
            flo, fhi = FG[g]
            blo, bhi = BG[g]
            fw = (fhi - flo) * BL
            bw = (bhi - blo) * BL
            # fwd round 0: segment 0 starts at step 1, others at step 0
            gchunks = range((flo * n) // CH, min(nchunks, (fhi * n - 1) // CH + 1))
            f0 = flo if flo > 0 else 1
            if f0 < fhi:
                w0 = (fhi - f0) * BL
                q0 = ps_q.tile([T, w0], FP32, space="PSUM", tag=f"q_f{g}")
                mm_banked(q0[:], E_bf, uw[:, f0 * BL:fhi * BL], w0)
                tt0 = nc.vector.tensor_tensor(
                    out=uw[:, f0 * BL:fhi * BL].rearrange("p (s b) -> p s b", b=BL),
                    in0=q0[:].rearrange("p (s b) -> p s b", b=BL),
                    in1=x4[:, f0:fhi, :, 0], op=Mult,
                )
                for c in gchunks:
                    for h in range(2):
                        add_dep_helper(tt0.ins, xp_insts[(c, h)].ins,
                                       reason="x ready")
            # bwd probes for middle segments only need ~NB contraction
            # steps (error ~0.1^NB): the fwd probe carries the segment
            # scale; gamma normalizes rho's arbitrary scale exactly.
            NB = 15
            for r in range(1, n):
                qf = ps_q.tile([T, fw], FP32, space="PSUM", tag=f"q_f{g}")
                mm_banked(qf[:], E_bf, uw[:, flo * BL:fhi * BL], fw)
                ttf = nc.vector.tensor_tensor(
                    out=uw[:, flo * BL:fhi * BL].rearrange(
                        "p (s b) -> p s b", b=BL
                    ),
                    in0=qf[:].rearrange("p (s b) -> p s b", b=BL),
                    in1=x4[:, flo:fhi, :, r], op=Mult,
                )
                if r == 1:
                    for c in gchunks:
                        for h in range(2):
                            add_dep_helper(ttf.ins, xp_insts[(c, h)].ins,
                                           reason="x ready")
                rb = r - 1
                if rb < NB:
                    lo2, hi2 = blo, bhi
                elif bhi == NP and rb < n - 1:
                    lo2, hi2 = NP - 1, NP  # segment S-1 runs the full length
                else:
                    continue
                bw2 = (hi2 - lo2) * BL
                qb = ps_q.tile([T, bw2], FP32, space="PSUM", tag=f"q_b{g}")
                mm_banked(qb[:], ET_bf, uw[:, W + lo2 * BL:W + hi2 * BL], bw2)
                ttb = nc.vector.tensor_tensor(
                    out=uw[:, W + lo2 * BL:W + hi2 * BL].rearrange(
                        "p (s b) -> p s b", b=BL
                    ),
                    in0=qb[:].rearrange("p (s b) -> p s b", b=BL),
                    in1=x4[:, lo2 + 1:hi2 + 1, :, n - 2 - rb], op=Mult,
                )
                if rb == 0:
                    for c in gchunks:
                        for h in range(2):
                            add_dep_helper(ttb.ins, xp_insts[(c, h)].ins,
                                           reason="x ready")
                if rb == NB - 1:
                    # emit rho for the short slots now (all but seg S-1 slot)
                    shi = bhi - 1 if bhi == NP else bhi
                    if shi > blo:
                        rw = (shi - blo) * BL
                        rho = ps_q.tile([T, rw], FP32, space="PSUM",
                                        tag=f"q_b{g}")
                        mm_banked(rho[:], ET_bf, uw[:, W + blo * BL:W + shi * BL], rw)
                        nc.vector.tensor_copy(
                            out=rho_sb[:, blo * BL:shi * BL], in_=rho[:]
                        )
            if bhi == NP:
                rho = ps_q.tile([T, BL], FP32, space="PSUM", tag=f"q_b{g}")
                mm_banked(rho[:], ET_bf, uw[:, W + (NP - 1) * BL:W + NP * BL], BL)
                nc.vector.tensor_copy(
                    out=rho_sb[:, (NP - 1) * BL:W], in_=rho[:]
                )

        # ---------------- combine ----------------
        # d_s = rho_s . u_{s-1} (slots aligned); gamma_s = sum_t rho_s
        prod = sm_pool.tile([T, W], FP32, tag="prod")
        nc.vector.tensor_tensor(out=prod[:], in0=rho_sb[:], in1=uw[:, 0:W], op=Mult)
        drow_ps = ps_misc.tile([1, W], FP32, space="PSUM", tag="misc")
        mm_banked(drow_ps[:], ones_col_f32, prod[:], W)
        ln_d = sm_pool.tile([1, W], FP32, tag="ln_d")
        nc.scalar.activation(out=ln_d[:], in_=drow_ps[:], func=Ln)
        grow_ps = ps_misc.tile([1, (S - 2) * BL], FP32, space="PSUM", tag="misc")
        mm_banked(grow_ps[:], ones_col_f32, rho_sb[:, 0:(S - 2) * BL], (S - 2) * BL)
        ln_g = sm_pool.tile([1, (S - 2) * BL], FP32, tag="ln_g")
        nc.scalar.activation(out=ln_g[:], in_=grow_ps[:], func=Ln)
        zred = sm_pool.tile([1, BL], FP32, tag="zred")
        nc.vector.tensor_reduce(
            out=zred[:], in_=ln_d[:].rearrange("p (s b) -> p b s", b=BL),
            axis=mybir.AxisListType.X, op=Add,
        )
        gred = sm_pool.tile([1, BL], FP32, tag="gred")
        nc.vector.tensor_reduce(
            out=gred[:], in_=ln_g[:].rearrange("p (s b) -> p b s", b=BL),
            axis=mybir.AxisListType.X, op=Add,
        )
        z_row = sm_pool.tile([1, BL], FP32, tag="z_row")
        nc.vector.tensor_tensor(out=z_row[:], in0=zred[:], in1=gred[:], op=Sub)
        nc.vector.tensor_scalar(
            out=z_row[:], in0=z_row[:], scalar1=float((L - 1) * 7 * np.log(2.0)),
            scalar2=None, op0=Add,
        )
        nc.sync.dma_start(out=out_z[:], in_=z_row[:])

    # Postamble: drain + clear semaphores so the NEFF is re-executable
    # (without target_bir_lowering there is no preamble sem_clear).
    nc.reset()
    return nc


def _split_multi_waits(nc):
    """Workaround: this walrus encodes at most ONE sync-wait per instruction
    ("Too many sync wait commands"). Move extra waits onto same-engine NoOps
    inserted immediately before the instruction (engine blocks on each in
    program order, so semantics are identical)."""
    for fn in nc.m.functions:
        for bb in fn.blocks:
            insts = bb.instructions
            i = 0
            while i < len(insts):
                inst = insts[i]
                si = inst.sync_info
                if si is not None and si.on_wait and len(si.on_wait) > 1:
                    waits = list(si.on_wait)
                    for k, wsync in enumerate(waits[:-1]):
                        nop = mybir.InstNoOp(
                            name=f"{inst.name}-w{k}",
                            engine=inst.engine,
                            ins=[],
                            outs=[],
                            sync_info=mybir.SyncInfo(on_wait=[wsync], on_update=[]),
                        )
                        insts.insert(i, nop)
                        i += 1
                    inst.sync_info = mybir.SyncInfo(
                        on_wait=[waits[-1]], on_update=list(si.on_update or [])
                    )
                i += 1
    return nc


_NC_CACHE = {}


def _get_nc():
    key = "full"
    if key not in _NC_CACHE:
        builder = (
            build_crf_kernel_v2
            if int(os.environ.get("CRF_V2", "1"))
            else build_crf_kernel
        )
        _NC_CACHE[key] = _split_multi_waits(builder())
    return _NC_CACHE[key]


def make_in_maps(emissions, tags, start_transitions, end_transitions, transitions):
    emissions = np.ascontiguousarray(np.asarray(emissions, dtype=np.float32))
    tags = np.ascontiguousarray(np.asarray(tags).astype(np.int32))
    start = np.ascontiguousarray(
        np.asarray(start_transitions, dtype=np.float32).reshape(T, 1)
    )
    end = np.ascontiguousarray(
        np.asarray(end_transitions, dtype=np.float32).reshape(T, 1)
    )
    trans = np.ascontiguousarray(np.asarray(transitions, dtype=np.float32))
    in_maps = []
    for i in range(NCORES):
        sl = slice(i * BL, (i + 1) * BL)
        in_maps.append({
            "emissions": np.ascontiguousarray(emissions[:, sl, :]),
            "tags": np.ascontiguousarray(tags[:, sl]),
            "start_t": start,
            "end_t": end,
            "trans": trans,
        })
    return in_maps


def combine_outputs(results):
    log_den = 0.0
    log_num = 0.0
    for res in results:
        log_den += np.asarray(res["out_z"], dtype=np.float64).sum()
        log_num += np.asarray(res["out_gold"], dtype=np.float64).sum()
        log_num += np.asarray(res["out_trans"], dtype=np.float64).sum()
        log_num += np.asarray(res["out_se"], dtype=np.float64).sum()
    return np.float32((log_den - log_num) / B)


def kernel(emissions, tags, mask, start_transitions, end_transitions, transitions):
    mask = np.asarray(mask)
    assert mask.all(), "kernel assumes mask of all ones (spec fill=ones)"
    from concourse.bass_utils import run_bass_kernel_spmd

    nc = _get_nc()
    in_maps = make_in_maps(
        emissions, tags, start_transitions, end_transitions, transitions
    )
    trace = bool(int(os.environ.get("CRF_TRACE", "0")))
    if trace:
        try:
            import importlib.util as _iu

            try:
                from antenv import axon_hooks as _hooks
            except ImportError:
                # antenv may already be cached from a copy lacking axon_hooks;
                # load ours by path and graft it into the package.
                import antenv

                _spec = _iu.spec_from_file_location(
                    "antenv.axon_hooks", "/opt/trn_rl_repo/antenv/axon_hooks.py"
                )
                _hooks = _iu.module_from_spec(_spec)
                _spec.loader.exec_module(_hooks)
                sys.modules["antenv.axon_hooks"] = _hooks
                antenv.axon_hooks = _hooks

            if _hooks.get_axon_ntff_profile_hook() is None:
                from trn_agent_boot.trn_boot import _ntff_profile_via_ctypes

                _hooks.set_axon_ntff_profile_hook(
                    _ntff_profile_via_ctypes("/opt/axon/libaxon_pjrt.so")
                )
        except Exception as e:  # profiling is best-effort
            print(f"NTFF hook install failed ({e}); running untraced")
            trace = False
    if kernel._calls > 0:
        # Re-executing a cached NEFF is unreliable (semaphore state);
        # force a fresh PJRT executable (NEFF compile is disk-cached).
        import jax

        jax.clear_caches()
    kernel._calls += 1
    br = run_bass_kernel_spmd(nc, in_maps, list(range(NCORES)), trace=trace)
    if trace and br.exec_time_ns is not None:
        print(f"HW exec time: {br.exec_time_ns} ns")
        kernel.last_exec_time_ns = br.exec_time_ns
    return combine_outputs(br.results)


kernel.last_exec_time_ns = None
kernel._calls = 0
